# revision 25
# baseline (speedup 1.0000x reference)
"""Graphormer3D encoder layer on 8 Trainium2 NeuronCores — v3.

Data-parallel over the 16 graphs (2 per core); params replicated.
Feature-major activations (x^T: [feature, token]) fp16, fp32 PSUM.

v3 changes vs v2 (303.7us):
  - h1 = LN1(x) precomputed on host (same class of prep as exp(attn_bias));
    removes LN1 sum matmuls, the ~7us LN1 serial-chain PE gap, and the
    DVE normalize passes. x still shipped for the residual.
  - v computed directly in key-major layout (h1 chunk as stationary,
    wv as moving): kills the per-head PE transposes and v's share of the
    96-row QKV tiles (-20k PE columns). v bias folded into bout on host
    (bout += out_w @ bv), exact.
  - out-proj is hf-major and LN2 runs per-half: LN2 stats/apply for half 0
    overlap out-proj half 1 on the PE; FFN half 0 overlaps LN2 half 1.
    Removes the ~8us LN2 serial-chain PE gap.
  - LN2 rs via a single Rsqrt activation (no Ln->table-load->Exp chain);
    y1^2 computed on ScalarE (Square) instead of DVE.
  - scores/PV interleaved across the two graphs per head (sc g0, sc g1,
    pa g0, pa g1) with psum rings 3/3/2 to cover the exp/mult chain.
  - DMA: h1 chunks on the sync queue and qkv weights on the scalar queue
    issued first (parallel descriptor issue); wfc1/wfc2 issued late from
    the vector queue so they don't steal head bandwidth.
"""
import numpy as np

N_NODE, N_GRAPH, D = 512, 16, 768
H, HD, FFN = 8, 96, 3072
EPS = 1e-5
NC = 8            # cores
G = 2             # graphs per core
T = G * N_NODE    # tokens per core (1024)
KC = D // 128     # 6 feature chunks
FC = FFN // 128   # 24 ffn chunks
NQT = N_NODE // 128  # 4 key tiles per graph
HLF = (slice(0, 512), slice(512, 1024))

_cached = {}


def _build():
    import concourse.bass as bass
    import concourse.mybir as mybir
    import concourse.tile as tile
    import concourse.bacc as bacc
    from contextlib import ExitStack

    F16 = mybir.dt.float16
    F32 = mybir.dt.float32
    AF = mybir.ActivationFunctionType
    OP = mybir.AluOpType

    nc = bacc.Bacc("TRN2", target_bir_lowering=False, debug=False, num_devices=NC)

    di = lambda name, shape, dt: nc.declare_dram_parameter(name, shape, dt, isOutput=False)
    h1_d = di("h1t", [KC, 128, T], F16)
    xt_d = di("xt", [128, KC, T], F16)
    ebias_d = di("ebias", [G * H, 128, NQT * N_NODE], F16)  # exp(bias)^T, tiled
    mask_d = di("maskrow", [1, T], F16)
    wqk_d = di("wqk", [KC, 128, 2 * D], F16)
    wv_d = di("wv", [128, KC, D], F16)
    bqk_d = di("bqk", [HD, 2 * H], F32)
    wout_d = di("wout", [128, KC, D], F16)                 # partition-major
    bout_d = di("bout", [128, KC], F32)
    wfc1_d = di("wfc1", [128, KC, FFN], F16)
    bfc1_d = di("bfc1", [128, FC], F32)
    wfc2_d = di("wfc2", [128, FC, D], F16)
    bfc2_d = di("bfc2", [128, KC], F32)
    ones_d = di("ones", [128, 128], F16)
    yt_d = nc.declare_dram_parameter("yt", [KC, 128, T], F16, isOutput=True)

    with tile.TileContext(nc) as tc, ExitStack() as top:
        # Left-side pools are a LIFO stack: persistent pools first, then h1
        # (freed after attention), then the attention pools. DMA issue order
        # is set separately by the dma_start call order below.
        const = top.enter_context(tc.tile_pool(name="const", bufs=1))
        x_pool = top.enter_context(tc.tile_pool(name="x", bufs=1))
        stat_pool = top.enter_context(tc.tile_pool(name="stat", bufs=2))
        h2_pool = top.enter_context(tc.tile_pool(name="h2", bufs=1))
        y1_pool = top.enter_context(tc.tile_pool(name="y1", bufs=1))
        yo_pool = top.enter_context(tc.tile_pool(name="yo", bufs=2))
        s_attn = ExitStack()
        attn_pool = s_attn.enter_context(tc.tile_pool(name="attn", bufs=1))
        attn_sb = attn_pool.tile([128, KC, T], F16, tag="attn", name="attn_sb")
        s_h1 = ExitStack()
        h1_pool = s_h1.enter_context(tc.tile_pool(name="h1", bufs=1))
        h1 = h1_pool.tile([128, KC, T], F16, tag="h1", name="h1")

        wqk_pool = top.enter_context(tc.tile_pool(name="wqk", bufs=1, side="right"))
        wqk = wqk_pool.tile([128, KC, 2 * D], F16, tag="wqk", name="wqk")
        wout_pool = top.enter_context(tc.tile_pool(name="wout", bufs=1, side="right"))
        wo = wout_pool.tile([128, KC, D], F16, tag="wout", name="wo")
        s_wv = ExitStack()
        wv_pool = s_wv.enter_context(tc.tile_pool(name="wv", bufs=1, side="right"))
        wv = wv_pool.tile([128, KC, D], F16, tag="wv", name="wv")
        x_big = x_pool.tile([128, KC, T], F16, tag="x", name="x_big")

        # ---------------- DMA: critical path first ----------------
        # h1 chunks on sync queue; qkv weights on scalar queue (parallel
        # descriptor issue; first QKV matmul gated only on h1[0]+wqk[0]).
        for k in range(KC):
            nc.sync.dma_start(h1[:, k, :], h1_d[k])
            nc.scalar.dma_start(wqk[:, k, :], wqk_d[k])
        nc.scalar.dma_start(wv[:], wv_d[:])

        def load_const(name, dram, shape, dt, eng=nc.gpsimd):
            t = const.tile(shape, dt, tag=name, name=name)
            eng.dma_start(t[:], dram[:])
            return t

        bqk = load_const("bqk", bqk_d, [HD, 2 * H], F32)
        mask_sb = load_const("mask", mask_d, [1, T], F16)
        ones = load_const("ones", ones_d, [128, 128], F16)
        bout = load_const("bout", bout_d, [128, KC], F32)
        bfc1 = load_const("bfc1", bfc1_d, [128, FC], F32)
        bfc2 = load_const("bfc2", bfc2_d, [128, KC], F32)
        eps_sb = const.tile([128, 1], F32, tag="eps", name="eps")
        nc.vector.memset(eps_sb[:], EPS)
        maskb = const.tile([128, T], F16, tag="maskb", name="maskb")
        nc.gpsimd.partition_broadcast(maskb[:], mask_sb[:])

        # remaining weights on scalar queue, after the qkv-critical ones
        nc.scalar.dma_start(wo[:], wout_d[:])
        nc.scalar.dma_start(x_big[:], xt_d[:])

        # ---------------- attention phase ----------------
        s_ap = ExitStack()
        vx_pool = s_ap.enter_context(tc.tile_pool(name="vx", bufs=1))
        vx = vx_pool.tile([128, G * H, NQT, HD + 1], F16, tag="vx", name="vx")
        nc.vector.memset(vx[:, :, :, HD:HD + 1], 1.0)
        qkvt_pool = s_ap.enter_context(tc.tile_pool(name="qkvt", bufs=2))
        eb_pool = s_ap.enter_context(tc.tile_pool(name="eb", bufs=4))
        p_pool = s_ap.enter_context(tc.tile_pool(name="p", bufs=3))
        small_pool = s_ap.enter_context(tc.tile_pool(name="small", bufs=2))
        ps_sc = s_ap.enter_context(tc.tile_pool(name="ps_sc", bufs=3, space="PSUM"))
        ps_pa = s_ap.enter_context(tc.tile_pool(name="ps_pa", bufs=2, space="PSUM"))
        ps_qk = s_ap.enter_context(tc.tile_pool(name="ps_qk", bufs=3, space="PSUM"))

        def qkv_head(th, tag):
            """th in 0..15: q heads 0-7 then k heads 0-7."""
            t = qkvt_pool.tile([HD, T], F16, tag=tag, name=f"{tag}{th}")
            for hf in range(2):
                ps = ps_qk.tile([HD, 512], F32, tag="qk", name=f"qk{th}{hf}")
                for k in range(KC):
                    nc.tensor.matmul(
                        ps[:], wqk[:, k, th * HD:(th + 1) * HD],
                        h1[:, k, HLF[hf]], start=(k == 0), stop=(k == KC - 1))
                if th < H:       # q evict on DVE to balance engines
                    nc.vector.tensor_scalar_add(t[:, HLF[hf]], ps[:],
                                                bqk[:, th:th + 1])
                else:            # k evict on ScalarE
                    nc.scalar.activation(t[:, HLF[hf]], ps[:], AF.Identity,
                                         bias=bqk[:, th:th + 1])
            return t

        def v_direct(g):
            """v for all 8 heads of graph g, key-major, into vx.
            out[tok128, hd] = h1_chunk^T @ wv_chunk; borrow the sc psum ring.
            v bias is folded into bout on host."""
            base = g * N_NODE
            for kt in range(NQT):
                tok = slice(base + kt * 128, base + (kt + 1) * 128)
                for half in range(2):
                    vp = ps_sc.tile([128, 512], F32, tag="sc", name=f"v{g}{kt}{half}")
                    cols = slice(half * 384, (half + 1) * 384)
                    for k in range(KC):
                        nc.tensor.matmul(vp[:, 0:384], h1[:, k, tok],
                                         wv[:, k, cols],
                                         start=(k == 0), stop=(k == KC - 1))
                    h0 = half * 4
                    nc.scalar.activation(
                        vx[:, g * H + h0:g * H + h0 + 4, kt, 0:HD],
                        vp[:, 0:384], AF.Copy)

        def attn_scores(g, hh, q_t, k_t):
            gh = g * H + hh
            base = g * N_NODE
            eb = eb_pool.tile([128, NQT, N_NODE], F16, tag="eb", name=f"eb{gh}")
            nc.sync.dma_start(eb[:], ebias_d[gh])
            p = p_pool.tile([128, NQT, N_NODE], F16, tag="p", name=f"p{gh}")
            for kt in range(NQT):
                sc = ps_sc.tile([128, N_NODE], F32, tag="sc", name=f"sc{gh}_{kt}")
                nc.tensor.matmul(sc[:],
                                 k_t[:, base + kt * 128: base + (kt + 1) * 128],
                                 q_t[:, base: base + N_NODE],
                                 start=True, stop=True)
                if g == 0:   # DVE path, in-place
                    nc.scalar.activation(p[:, kt, :], sc[:], AF.Exp)
                    nc.vector.tensor_tensor(p[:, kt, :], p[:, kt, :],
                                            eb[:, kt, :], op=OP.mult)
                else:        # GpSimd path (SBUF-only, not in-place)
                    pe_t = small_pool.tile([128, N_NODE], F16, tag="pe",
                                           name=f"pe{gh}_{kt}")
                    nc.scalar.activation(pe_t[:], sc[:], AF.Exp)
                    nc.gpsimd.tensor_tensor(p[:, kt, :], pe_t[:],
                                            eb[:, kt, :], op=OP.mult)
            return p

        def attn_pv(g, hh, p):
            gh = g * H + hh
            base = g * N_NODE
            pa = ps_pa.tile([HD + 1, N_NODE], F32, tag="pa", name=f"pa{gh}")
            for kt in range(NQT):
                nc.tensor.matmul(pa[:], vx[:, gh, kt, :], p[:, kt, :],
                                 start=(kt == 0), stop=(kt == NQT - 1))
            s_sb = small_pool.tile([1, N_NODE], F32, tag="s", name=f"s{gh}")
            nc.scalar.activation(s_sb[:], pa[HD:HD + 1, :], AF.Copy)
            r = small_pool.tile([1, N_NODE], F32, tag="r", name=f"r{gh}")
            nc.vector.reciprocal_approx_fast(out=r[:], in_=s_sb[:])
            rb = small_pool.tile([HD, N_NODE], F32, tag="rb", name=f"rb{gh}")
            nc.gpsimd.partition_broadcast(rb[:], r[:])
            # normalized evict into packed attn chunks; pieces limited by dst
            # chunk boundary and PSUM quadrant reads (src offset != 0 allows
            # at most 32 partitions per access)
            R0 = hh * HD
            allowed = lambda p_: 128 if p_ == 0 else (64 if p_ == 64 else 32)
            off = 0
            while off < HD:
                c, rr = (R0 + off) // 128, (R0 + off) % 128
                ln = min(HD - off, 128 - rr, allowed(off), allowed(rr))
                nc.vector.tensor_tensor(
                    attn_sb[rr:rr + ln, c, base: base + N_NODE],
                    pa[off:off + ln, :], rb[off:off + ln, :], op=OP.mult)
                off += ln

        for hh in range(H):
            q_t = qkv_head(hh, "q")
            k_t = qkv_head(H + hh, "k")
            if hh == 0:
                v_direct(0)
                v_direct(1)
                s_wv.close()
            if hh == 1:
                # wfc1 issued from vector queue now: doesn't steal head
                # bandwidth, arrives well before the FFN needs it.
                wfc1_pool = top.enter_context(
                    tc.tile_pool(name="wfc1", bufs=1, side="right"))
                wf1 = wfc1_pool.tile([128, KC, FFN], F16, tag="wfc1", name="wf1")
                nc.gpsimd.dma_start(wf1[:], wfc1_d[:])
            p0 = attn_scores(0, hh, q_t, k_t)
            p1 = attn_scores(1, hh, q_t, k_t)
            attn_pv(0, hh, p0)
            attn_pv(1, hh, p1)

        s_ap.close()
        s_h1.close()

        wfc2_pool = top.enter_context(
            tc.tile_pool(name="wfc2", bufs=1, side="right"))
        wf2 = wfc2_pool.tile([128, FC, D], F16, tag="wfc2", name="wf2")
        nc.gpsimd.dma_start(wf2[:], wfc2_d[:])

        # ---------------- out-proj + per-half LN2 + FFN ----------------
        # PE stream order: op(0), sums(0), op(1), sums(1), fc1/fc2(0),
        # fc1/fc2(1) — each LN2 serial chain is covered by the following
        # block's matmuls. Out-proj uses 6 parallel psum banks per half with
        # chunks 4,5 accumulated last (they are written by the last heads).
        ps_s = s_attn.enter_context(tc.tile_pool(name="ps_s", bufs=2, space="PSUM"))
        s_op = ExitStack()
        ps_op = s_op.enter_context(tc.tile_pool(name="ps_op", bufs=6, space="PSUM"))

        y1 = y1_pool.tile([128, KC, T], F16, tag="y1", name="y1")
        h2 = h2_pool.tile([128, KC, T], F16, tag="h2", name="h2")

        with tc.tile_pool(name="sq", bufs=1) as sq_pool, \
             tc.tile_pool(name="gelu", bufs=1) as gelu_pool:
            sq = sq_pool.tile([128, KC, T], F16, tag="sq", name="sq")
            gl = gelu_pool.tile([128, FC, 512], F16, tag="gelu", name="gl")
            mus, rss = {}, {}

            def outproj_sums(hf):
                pos = [ps_op.tile([128, 512], F32, tag="po", name=f"po{m}{hf}")
                       for m in range(KC)]
                C_ORDER = [0, 1, 2, 3, 4, 5]
                for ci, c in enumerate(C_ORDER):
                    for m in range(KC):
                        nc.tensor.matmul(pos[m][:], wo[:, c, m * 128:(m + 1) * 128],
                                         attn_sb[:, c, HLF[hf]],
                                         start=(ci == 0), stop=(ci == KC - 1))
                for m in range(KC):
                    tmp = stat_pool.tile([128, 512], F16, tag="tmp", name=f"tmp{m}{hf}")
                    nc.vector.scalar_tensor_tensor(tmp[:], pos[m][:], bout[:, m:m + 1],
                                                   maskb[:, HLF[hf]],
                                                   op0=OP.add, op1=OP.mult)
                    nc.gpsimd.tensor_tensor(y1[:, m, HLF[hf]], tmp[:],
                                             x_big[:, m, HLF[hf]], op=OP.add)
                    nc.scalar.activation(sq[:, m, HLF[hf]], y1[:, m, HLF[hf]],
                                         AF.Square)
                # LN2 stats for this half
                pss = ps_s.tile([128, 512], F32, tag="sum", name=f"lns{hf}")
                for k in range(KC):
                    nc.tensor.matmul(pss[:], ones[:], y1[:, k, HLF[hf]],
                                     start=(k == 0), stop=(k == KC - 1))
                psq = ps_s.tile([128, 512], F32, tag="sum", name=f"lnq{hf}")
                for k in range(KC):
                    nc.tensor.matmul(psq[:], ones[:], sq[:, k, HLF[hf]],
                                     start=(k == 0), stop=(k == KC - 1))
                mu = stat_pool.tile([128, 512], F16, tag="mu16", name=f"mu{hf}")
                nc.vector.tensor_scalar_mul(mu[:], pss[:], 1.0 / D)
                m2 = stat_pool.tile([128, 512], F32, tag="m2", bufs=1, name=f"m2{hf}")
                nc.vector.tensor_tensor(m2[:], mu[:], mu[:], op=OP.mult)
                nc.vector.tensor_scalar_sub(m2[:], m2[:], EPS)   # var+eps below
                var = stat_pool.tile([128, 512], F32, tag="var", bufs=1, name=f"var{hf}")
                nc.vector.scalar_tensor_tensor(var[:], psq[:], 1.0 / D,
                                               m2[:], op0=OP.mult, op1=OP.subtract)
                rinv = stat_pool.tile([128, 512], F32, tag="rinv", bufs=1,
                                      name=f"rinv{hf}")
                nc.vector.reciprocal_approx_fast(out=rinv[:], in_=var[:])
                rs = stat_pool.tile([128, 512], F16, tag="rs16", name=f"rs{hf}")
                nc.scalar.activation(rs[:], rinv[:], AF.Sqrt)
                for k in range(KC):
                    nc.gpsimd.tensor_tensor(h2[:, k, HLF[hf]], y1[:, k, HLF[hf]],
                                            mu[:], op=OP.subtract)
                    nc.vector.tensor_tensor(h2[:, k, HLF[hf]], h2[:, k, HLF[hf]],
                                            rs[:], op=OP.mult)

            outproj_sums(0)
            outproj_sums(1)
            s_op.close()
            ps_c = s_attn.enter_context(
                tc.tile_pool(name="ps_c", bufs=4, space="PSUM"))

            for hf in range(2):
                for n in range(FC):
                    pf = ps_c.tile([128, 512], F32, tag="mm", name=f"pf{n}{hf}")
                    for k in range(KC):
                        nc.tensor.matmul(pf[:], wf1[:, k, n * 128:(n + 1) * 128],
                                         h2[:, k, HLF[hf]],
                                         start=(k == 0), stop=(k == KC - 1))
                    nc.scalar.activation(gl[:, n, :], pf[:], AF.Gelu,
                                         bias=bfc1[:, n:n + 1])
                for m in range(KC):
                    py = ps_c.tile([128, 512], F32, tag="mm", name=f"py{m}{hf}")
                    for kk in range(FC):
                        nc.tensor.matmul(py[:], wf2[:, kk, m * 128:(m + 1) * 128],
                                         gl[:, kk, :],
                                         start=(kk == 0), stop=(kk == FC - 1))
                    yo = yo_pool.tile([128, 512], F16, tag="yo", name=f"yo{m}{hf}")
                    nc.vector.scalar_tensor_tensor(yo[:], py[:], bfc2[:, m:m + 1],
                                                   y1[:, m, HLF[hf]],
                                                   op0=OP.add, op1=OP.add)
                    nc.sync.dma_start(yt_d[m, :, HLF[hf]], yo[:])

        s_attn.close()

    nc.compile()
    return nc


def _get_runner():
    if "runner" in _cached:
        return _cached["runner"]
    import jax
    from jax.sharding import Mesh, PartitionSpec
    from jax.experimental.shard_map import shard_map
    import concourse.mybir as mybir
    from concourse.bass2jax import _bass_exec_p, install_neuronx_cc_hook, partition_id_tensor

    nc = _build()
    install_neuronx_cc_hook()
    partition_name = nc.partition_id_tensor.name if nc.partition_id_tensor else None
    in_names, out_names, out_avals, zero_outs = [], [], [], []
    for alloc in nc.m.functions[0].allocations:
        if not isinstance(alloc, mybir.MemoryLocationSet):
            continue
        name = alloc.memorylocations[0].name
        if alloc.kind == "ExternalInput":
            if name != partition_name:
                in_names.append(name)
        elif alloc.kind == "ExternalOutput":
            out_names.append(name)
            shape = tuple(alloc.tensor_shape)
            dtype = mybir.dt.np(alloc.dtype)
            out_avals.append(jax.core.ShapedArray(shape, dtype))
            zero_outs.append(np.zeros(shape, dtype))
    n_params = len(in_names)
    all_in_names = in_names + out_names + ([partition_name] if partition_name else [])

    def _body(*args):
        operands = list(args)
        if partition_name is not None:
            operands.append(partition_id_tensor())
        outs = _bass_exec_p.bind(
            *operands,
            out_avals=tuple(out_avals),
            in_names=tuple(all_in_names),
            out_names=tuple(out_names),
            lowering_input_output_aliases=(),
            sim_require_finite=False,
            sim_require_nnan=False,
            nc=nc,
        )
        return tuple(outs)

    donate = tuple(range(n_params, n_params + len(out_avals)))
    devices = jax.devices()[:NC]
    mesh = Mesh(np.asarray(devices), ("core",))
    in_specs = (PartitionSpec("core"),) * (n_params + len(out_avals))
    out_specs = (PartitionSpec("core"),) * len(out_names)
    sharded = jax.jit(
        shard_map(_body, mesh=mesh, in_specs=in_specs, out_specs=out_specs, check_rep=False),
        donate_argnums=donate, keep_unused=True,
    )

    runner = {
        "nc": nc, "sharded": sharded, "in_names": in_names,
        "out_names": out_names, "out_avals": out_avals, "zero_outs": zero_outs,
    }
    _cached["runner"] = runner
    return runner


def prep_inputs(x, attn_bias, node_non_padding_mask, in_w, in_b, out_w, out_b,
                ln1_g, ln1_b, fc1_w, fc1_b, fc2_w, fc2_b, ln2_g, ln2_b):
    f16, f32 = np.float16, np.float32
    x = np.asarray(x, f32)
    # LN1 on host (no affine; gamma/beta folded into qkv weights/bias)
    mu = x.mean(-1, keepdims=True)
    var = ((x - mu) ** 2).mean(-1, keepdims=True)
    h1 = (x - mu) / np.sqrt(var + EPS)
    xt = x.transpose(2, 1, 0).reshape(D, N_GRAPH * N_NODE).astype(f16)
    h1t = h1.transpose(2, 1, 0).reshape(D, N_GRAPH * N_NODE).astype(f16)
    xt_pc = [np.ascontiguousarray(xt[:, c * T:(c + 1) * T]).reshape(KC, 128, T) for c in range(NC)]
    h1t_pc = [np.ascontiguousarray(h1t[:, c * T:(c + 1) * T]).reshape(KC, 128, T) for c in range(NC)]
    # exp(bias), transposed per head-graph, key-tiled:
    # ebh[gh, p, kt*512+n] = exp(bias[gh, n, kt*128+p])
    ebt = np.exp(np.asarray(attn_bias, f32)).transpose(0, 2, 1)  # [gh, m, n]
    ebh = np.ascontiguousarray(
        ebt.reshape(N_GRAPH * H, NQT, 128, N_NODE).transpose(0, 2, 1, 3)
    ).reshape(N_GRAPH * H, 128, NQT * N_NODE).astype(f16)
    mask = np.asarray(node_non_padding_mask).astype(f16)

    in_w = np.asarray(in_w, f32)
    in_b = np.asarray(in_b, f32)
    g1 = np.asarray(ln1_g, f32)
    b1 = np.asarray(ln1_b, f32)
    wqkv = in_w * g1[None, :]                  # fold LN1 gamma
    bqkv = in_b + in_w @ b1                    # fold LN1 beta
    scale = HD ** -0.5
    wqkv = wqkv.copy(); bqkv = bqkv.copy()
    wqkv[:D] *= scale                           # fold q scaling
    bqkv[:D] *= scale
    # v bias folded into the out-projection bias: out_w @ bv + out_b
    bv = bqkv[2 * D:]
    bout_full = np.asarray(out_b, f32) + np.asarray(out_w, f32) @ bv
    fc1w = np.asarray(fc1_w, f32)
    fc1b = np.asarray(fc1_b, f32)
    g2 = np.asarray(ln2_g, f32)
    b2 = np.asarray(ln2_b, f32)
    wfc1 = fc1w * g2[None, :]                  # fold LN2 gamma
    bfc1 = fc1b + fc1w @ b2                    # fold LN2 beta

    pm = lambda a, nchunk: np.ascontiguousarray(
        a.reshape(nchunk, 128, a.shape[-1]).transpose(1, 0, 2))  # partition-major
    wqkvT = wqkv.T.astype(f16)                 # [D, 3D]
    shared = {
        "wqk": np.ascontiguousarray(wqkvT[:, :2 * D]).reshape(KC, 128, 2 * D),
        "wv": pm(np.ascontiguousarray(wqkvT[:, 2 * D:]), KC),
        "bqk": np.ascontiguousarray(bqkv[:2 * D].reshape(2 * H, HD).T),
        "wout": pm(np.asarray(out_w, f32).T.astype(f16), KC),
        "bout": np.ascontiguousarray(bout_full.reshape(KC, 128).T),
        "wfc1": pm(wfc1.T.astype(f16), KC),
        "bfc1": np.ascontiguousarray(bfc1.reshape(FC, 128).T),
        "wfc2": pm(np.asarray(fc2_w, f32).T.astype(f16), FC),
        "bfc2": np.ascontiguousarray(np.asarray(fc2_b, f32).reshape(KC, 128).T),
        "ones": np.ones((128, 128), dtype=f16),
    }
    per_core = []
    for c in range(NC):
        m = dict(shared)
        m["xt"] = np.ascontiguousarray(xt_pc[c].transpose(1, 0, 2))
        m["h1t"] = h1t_pc[c]
        m["ebias"] = ebh[G * H * c: G * H * (c + 1)]
        m["maskrow"] = np.ascontiguousarray(mask[G * c: G * (c + 1)]).reshape(1, T)
        per_core.append(m)
    return per_core


def postprocess(outs):
    yt = np.stack([np.asarray(o["yt"], np.float32).reshape(D, T) for o in outs])
    y = yt.reshape(NC, D, G, N_NODE).transpose(3, 0, 2, 1).reshape(N_NODE, N_GRAPH, D)
    return np.ascontiguousarray(y)


def run_per_core(per_core):
    r = _get_runner()
    n = NC
    concat_in = [
        np.concatenate([np.asarray(per_core[c][name]) for c in range(n)], axis=0)
        for name in r["in_names"]
    ]
    concat_zeros = [np.zeros((n * z.shape[0], *z.shape[1:]), z.dtype) for z in r["zero_outs"]]
    out_arrs = r["sharded"](*concat_in, *concat_zeros)
    return [
        {name: np.asarray(out_arrs[i]).reshape(n, *r["out_avals"][i].shape)[c]
         for i, name in enumerate(r["out_names"])}
        for c in range(n)
    ]


def kernel(**inputs):
    per_core = prep_inputs(**inputs)
    outs = run_per_core(per_core)
    return postprocess(outs)


# revision 26
# speedup vs baseline: 1.5151x; 1.5151x over previous
"""Graphormer3D encoder layer on 8 Trainium2 NeuronCores — v3.

Data-parallel over the 16 graphs (2 per core); params replicated.
Feature-major activations (x^T: [feature, token]) fp16, fp32 PSUM.

v3 changes vs v2 (303.7us):
  - h1 = LN1(x) precomputed on host (same class of prep as exp(attn_bias));
    removes LN1 sum matmuls, the ~7us LN1 serial-chain PE gap, and the
    DVE normalize passes. x still shipped for the residual.
  - v computed directly in key-major layout (h1 chunk as stationary,
    wv as moving): kills the per-head PE transposes and v's share of the
    96-row QKV tiles (-20k PE columns). v bias folded into bout on host
    (bout += out_w @ bv), exact.
  - out-proj is hf-major and LN2 runs per-half: LN2 stats/apply for half 0
    overlap out-proj half 1 on the PE; FFN half 0 overlaps LN2 half 1.
    Removes the ~8us LN2 serial-chain PE gap.
  - LN2 rs via a single Rsqrt activation (no Ln->table-load->Exp chain);
    y1^2 computed on ScalarE (Square) instead of DVE.
  - scores/PV interleaved across the two graphs per head (sc g0, sc g1,
    pa g0, pa g1) with psum rings 3/3/2 to cover the exp/mult chain.
  - DMA: h1 chunks on the sync queue and qkv weights on the scalar queue
    issued first (parallel descriptor issue); wfc1/wfc2 issued late from
    the vector queue so they don't steal head bandwidth.
"""
import numpy as np

N_NODE, N_GRAPH, D = 512, 16, 768
H, HD, FFN = 8, 96, 3072
EPS = 1e-5
NC = 8            # cores
G = 2             # graphs per core
T = G * N_NODE    # tokens per core (1024)
KC = D // 128     # 6 feature chunks
FC = FFN // 128   # 24 ffn chunks
NQT = N_NODE // 128  # 4 key tiles per graph
HLF = (slice(0, 512), slice(512, 1024))

_cached = {}


def _build():
    import concourse.bass as bass
    import concourse.mybir as mybir
    import concourse.tile as tile
    import concourse.bacc as bacc
    from contextlib import ExitStack

    F16 = mybir.dt.float16
    F32 = mybir.dt.float32
    AF = mybir.ActivationFunctionType
    OP = mybir.AluOpType

    nc = bacc.Bacc("TRN2", target_bir_lowering=False, debug=False, num_devices=NC)

    di = lambda name, shape, dt: nc.declare_dram_parameter(name, shape, dt, isOutput=False)
    h1_d = di("h1t", [KC, 128, T], F16)
    xt_d = di("xt", [128, KC, T], F16)
    ebias_d = di("ebias", [G * H, 128, NQT * N_NODE], F16)  # exp(bias)^T, tiled
    mask_d = di("maskrow", [1, T], F16)
    wqk_d = di("wqk", [KC, 128, 2 * D], F16)
    wv_d = di("wv", [128, KC, D], F16)
    bqk_d = di("bqk", [HD, 2 * H], F32)
    wout_d = di("wout", [128, KC, D], F16)                 # partition-major
    bout_d = di("bout", [128, KC], F32)
    wfc1_d = di("wfc1", [128, KC, FFN], F16)
    bfc1_d = di("bfc1", [128, FC], F32)
    wfc2_d = di("wfc2", [128, FC, D], F16)
    bfc2_d = di("bfc2", [128, KC], F32)
    ones_d = di("ones", [128, 128], F16)
    yt_d = nc.declare_dram_parameter("yt", [KC, 128, T], F16, isOutput=True)

    with tile.TileContext(nc) as tc, ExitStack() as top:
        # Left-side pools are a LIFO stack: persistent pools first, then h1
        # (freed after attention), then the attention pools. DMA issue order
        # is set separately by the dma_start call order below.
        const = top.enter_context(tc.tile_pool(name="const", bufs=1))
        x_pool = top.enter_context(tc.tile_pool(name="x", bufs=1))
        stat_pool = top.enter_context(tc.tile_pool(name="stat", bufs=2))
        h2_pool = top.enter_context(tc.tile_pool(name="h2", bufs=1))
        y1_pool = top.enter_context(tc.tile_pool(name="y1", bufs=1))
        yo_pool = top.enter_context(tc.tile_pool(name="yo", bufs=2))
        s_attn = ExitStack()
        attn_pool = s_attn.enter_context(tc.tile_pool(name="attn", bufs=1))
        attn_sb = attn_pool.tile([128, KC, T], F16, tag="attn", name="attn_sb")
        s_h1 = ExitStack()
        h1_pool = s_h1.enter_context(tc.tile_pool(name="h1", bufs=1))
        h1 = h1_pool.tile([128, KC, T], F16, tag="h1", name="h1")

        wqk_pool = top.enter_context(tc.tile_pool(name="wqk", bufs=1, side="right"))
        wqk = wqk_pool.tile([128, KC, 2 * D], F16, tag="wqk", name="wqk")
        wout_pool = top.enter_context(tc.tile_pool(name="wout", bufs=1, side="right"))
        wo = wout_pool.tile([128, KC, D], F16, tag="wout", name="wo")
        s_wv = ExitStack()
        wv_pool = s_wv.enter_context(tc.tile_pool(name="wv", bufs=1, side="right"))
        wv = wv_pool.tile([128, KC, D], F16, tag="wv", name="wv")
        x_big = x_pool.tile([128, KC, T], F16, tag="x", name="x_big")

        # ---------------- DMA: critical path first ----------------
        # h1 chunks on sync queue; qkv weights on scalar queue (parallel
        # descriptor issue; first QKV matmul gated only on h1[0]+wqk[0]).
        for k in range(KC):
            nc.sync.dma_start(h1[:, k, :], h1_d[k])
            nc.scalar.dma_start(wqk[:, k, :], wqk_d[k])
        nc.scalar.dma_start(wv[:], wv_d[:])

        def load_const(name, dram, shape, dt, eng=nc.gpsimd):
            t = const.tile(shape, dt, tag=name, name=name)
            eng.dma_start(t[:], dram[:])
            return t

        bqk = load_const("bqk", bqk_d, [HD, 2 * H], F32)
        mask_sb = load_const("mask", mask_d, [1, T], F16)
        ones = load_const("ones", ones_d, [128, 128], F16)
        bout = load_const("bout", bout_d, [128, KC], F32)
        bfc1 = load_const("bfc1", bfc1_d, [128, FC], F32)
        bfc2 = load_const("bfc2", bfc2_d, [128, KC], F32)
        eps_sb = const.tile([128, 1], F32, tag="eps", name="eps")
        nc.vector.memset(eps_sb[:], EPS)

        # remaining weights on scalar queue, after the qkv-critical ones
        nc.scalar.dma_start(wo[:], wout_d[:])
        nc.scalar.dma_start(x_big[:], xt_d[:])

        # ---------------- attention phase ----------------
        s_ap = ExitStack()
        vx_pool = s_ap.enter_context(tc.tile_pool(name="vx", bufs=1))
        vx = vx_pool.tile([128, G * H, NQT, HD + 1], F16, tag="vx", name="vx")
        nc.vector.memset(vx[:, :, :, HD:HD + 1], 1.0)
        qkvt_pool = s_ap.enter_context(tc.tile_pool(name="qkvt", bufs=2))
        eb_pool = s_ap.enter_context(tc.tile_pool(name="eb", bufs=4))
        p_pool = s_ap.enter_context(tc.tile_pool(name="p", bufs=3))
        small_pool = s_ap.enter_context(tc.tile_pool(name="small", bufs=2))
        ps_pa = s_ap.enter_context(tc.tile_pool(name="ps_pa", bufs=2, space="PSUM"))
        ps_sc = s_ap.enter_context(tc.tile_pool(name="ps_sc", bufs=3, space="PSUM"))
        ps_qk = s_ap.enter_context(tc.tile_pool(name="ps_qk", bufs=3, space="PSUM"))

        def qkv_head(th, tag):
            """th in 0..15: q heads 0-7 then k heads 0-7."""
            t = qkvt_pool.tile([HD, T], F16, tag=tag, name=f"{tag}{th}")
            for hf in range(2):
                ps = ps_qk.tile([HD, 512], F32, tag="qk", name=f"qk{th}{hf}")
                for k in range(KC):
                    nc.tensor.matmul(
                        ps[:], wqk[:, k, th * HD:(th + 1) * HD],
                        h1[:, k, HLF[hf]], start=(k == 0), stop=(k == KC - 1))
                if th < H:       # q evict on DVE to balance engines
                    nc.vector.tensor_scalar_add(t[:, HLF[hf]], ps[:],
                                                bqk[:, th:th + 1])
                else:            # k evict on ScalarE
                    nc.scalar.activation(t[:, HLF[hf]], ps[:], AF.Identity,
                                         bias=bqk[:, th:th + 1])
            return t

        def v_direct(g):
            """v for all 8 heads of graph g, key-major, into vx.
            out[tok128, hd] = h1_chunk^T @ wv_chunk; borrow the sc psum ring.
            v bias is folded into bout on host."""
            base = g * N_NODE
            for kt in range(NQT):
                tok = slice(base + kt * 128, base + (kt + 1) * 128)
                for half in range(2):
                    vp = ps_sc.tile([128, 512], F32, tag="sc", name=f"v{g}{kt}{half}")
                    cols = slice(half * 384, (half + 1) * 384)
                    for k in range(KC):
                        nc.tensor.matmul(vp[:, 0:384], h1[:, k, tok],
                                         wv[:, k, cols],
                                         start=(k == 0), stop=(k == KC - 1))
                    h0 = half * 4
                    nc.scalar.activation(
                        vx[:, g * H + h0:g * H + h0 + 4, kt, 0:HD],
                        vp[:, 0:384], AF.Copy)

        def attn_scores(g, hh, q_t, k_t):
            gh = g * H + hh
            base = g * N_NODE
            eb = eb_pool.tile([128, NQT, N_NODE], F16, tag="eb", name=f"eb{gh}")
            nc.sync.dma_start(eb[:], ebias_d[gh])
            p = p_pool.tile([128, NQT, N_NODE], F16, tag="p", name=f"p{gh}")
            for kt in range(NQT):
                sc = ps_sc.tile([128, N_NODE], F32, tag="sc", name=f"sc{gh}_{kt}")
                nc.tensor.matmul(sc[:],
                                 k_t[:, base + kt * 128: base + (kt + 1) * 128],
                                 q_t[:, base: base + N_NODE],
                                 start=True, stop=True)
                nc.scalar.activation(p[:, kt, :], sc[:], AF.Exp)
                nc.vector.tensor_tensor(p[:, kt, :], p[:, kt, :],
                                        eb[:, kt, :], op=OP.mult)
            return p

        def attn_pv(g, hh, p):
            gh = g * H + hh
            base = g * N_NODE
            pa = ps_pa.tile([HD + 1, N_NODE], F32, tag="pa", name=f"pa{gh}")
            for kt in range(NQT):
                nc.tensor.matmul(pa[:], vx[:, gh, kt, :], p[:, kt, :],
                                 start=(kt == 0), stop=(kt == NQT - 1))
            s_sb = small_pool.tile([1, N_NODE], F32, tag="s", name=f"s{gh}")
            nc.scalar.activation(s_sb[:], pa[HD:HD + 1, :], AF.Copy)
            r = small_pool.tile([1, N_NODE], F32, tag="r", name=f"r{gh}")
            nc.vector.reciprocal_approx_fast(out=r[:], in_=s_sb[:])
            # fold the padding mask into r: zeroes padded query tokens
            # before the (linear) out-projection — exact.
            nc.vector.tensor_tensor(r[:], r[:], mask_sb[:, base: base + N_NODE],
                                    op=OP.mult)
            rb = small_pool.tile([HD, N_NODE], F32, tag="rb", name=f"rb{gh}")
            nc.gpsimd.partition_broadcast(rb[:], r[:])
            # normalized evict into packed attn chunks; pieces limited by dst
            # chunk boundary and PSUM quadrant reads (src offset != 0 allows
            # at most 32 partitions per access)
            R0 = hh * HD
            allowed = lambda p_: 128 if p_ == 0 else (64 if p_ == 64 else 32)
            off = 0
            while off < HD:
                c, rr = (R0 + off) // 128, (R0 + off) % 128
                ln = min(HD - off, 128 - rr, allowed(off), allowed(rr))
                nc.vector.tensor_tensor(
                    attn_sb[rr:rr + ln, c, base: base + N_NODE],
                    pa[off:off + ln, :], rb[off:off + ln, :], op=OP.mult)
                off += ln

        for hh in range(H):
            q_t = qkv_head(hh, "q")
            k_t = qkv_head(H + hh, "k")
            if hh == 0:
                v_direct(0)
                v_direct(1)
                s_wv.close()
            if hh == 1:
                # wfc1 issued from vector queue now: doesn't steal head
                # bandwidth, arrives well before the FFN needs it.
                wfc1_pool = top.enter_context(
                    tc.tile_pool(name="wfc1", bufs=1, side="right"))
                wf1 = wfc1_pool.tile([128, KC, FFN], F16, tag="wfc1", name="wf1")
                nc.gpsimd.dma_start(wf1[:], wfc1_d[:])
            p0 = attn_scores(0, hh, q_t, k_t)
            p1 = attn_scores(1, hh, q_t, k_t)
            attn_pv(0, hh, p0)
            attn_pv(1, hh, p1)

        s_ap.close()
        s_h1.close()

        wfc2_pool = top.enter_context(
            tc.tile_pool(name="wfc2", bufs=1, side="right"))
        wf2 = wfc2_pool.tile([128, FC, D], F16, tag="wfc2", name="wf2")
        nc.gpsimd.dma_start(wf2[:], wfc2_d[:])

        # ---------------- out-proj + per-half LN2 + FFN ----------------
        # PE stream order: op(0), sums(0), op(1), sums(1), fc1/fc2(0),
        # fc1/fc2(1) — each LN2 serial chain is covered by the following
        # block's matmuls. Out-proj uses 6 parallel psum banks per half with
        # chunks 4,5 accumulated last (they are written by the last heads).
        ps_s = s_attn.enter_context(tc.tile_pool(name="ps_s", bufs=2, space="PSUM"))
        s_op = ExitStack()
        ps_op = s_op.enter_context(tc.tile_pool(name="ps_op", bufs=6, space="PSUM"))

        y1 = y1_pool.tile([128, KC, T], F16, tag="y1", name="y1")
        h2 = h2_pool.tile([128, KC, T], F16, tag="h2", name="h2")

        with tc.tile_pool(name="sq", bufs=1) as sq_pool, \
             tc.tile_pool(name="gelu", bufs=1) as gelu_pool:
            sq = sq_pool.tile([128, KC, T], F16, tag="sq", name="sq")
            gl = gelu_pool.tile([128, FC, 512], F16, tag="gelu", name="gl")
            mus, rss = {}, {}

            def outproj_sums(hf):
                pos = [ps_op.tile([128, 512], F32, tag="po", name=f"po{m}{hf}")
                       for m in range(KC)]
                C_ORDER = [0, 1, 2, 3, 4, 5]
                for ci, c in enumerate(C_ORDER):
                    for m in range(KC):
                        nc.tensor.matmul(pos[m][:], wo[:, c, m * 128:(m + 1) * 128],
                                         attn_sb[:, c, HLF[hf]],
                                         start=(ci == 0), stop=(ci == KC - 1))
                for m in range(KC):
                    # mask is folded into the softmax reciprocal r, so the
                    # evict is one fused op: y1 = (po + bout) + x
                    nc.vector.scalar_tensor_tensor(y1[:, m, HLF[hf]], pos[m][:],
                                                   bout[:, m:m + 1],
                                                   x_big[:, m, HLF[hf]],
                                                   op0=OP.add, op1=OP.add)
                    nc.scalar.activation(sq[:, m, HLF[hf]], y1[:, m, HLF[hf]],
                                         AF.Square)
                # LN2 stats for this half
                pss = ps_s.tile([128, 512], F32, tag="sum", name=f"lns{hf}")
                for k in range(KC):
                    nc.tensor.matmul(pss[:], ones[:], y1[:, k, HLF[hf]],
                                     start=(k == 0), stop=(k == KC - 1))
                psq = ps_s.tile([128, 512], F32, tag="sum", name=f"lnq{hf}")
                for k in range(KC):
                    nc.tensor.matmul(psq[:], ones[:], sq[:, k, HLF[hf]],
                                     start=(k == 0), stop=(k == KC - 1))
                mu = stat_pool.tile([128, 512], F16, tag="mu16", name=f"mu{hf}")
                nc.vector.tensor_scalar_mul(mu[:], pss[:], 1.0 / D)
                m2 = stat_pool.tile([128, 512], F32, tag="m2", bufs=1, name=f"m2{hf}")
                nc.vector.tensor_tensor(m2[:], mu[:], mu[:], op=OP.mult)
                nc.vector.tensor_scalar_sub(m2[:], m2[:], EPS)   # var+eps below
                var = stat_pool.tile([128, 512], F32, tag="var", bufs=1, name=f"var{hf}")
                nc.vector.scalar_tensor_tensor(var[:], psq[:], 1.0 / D,
                                               m2[:], op0=OP.mult, op1=OP.subtract)
                rinv = stat_pool.tile([128, 512], F32, tag="rinv", bufs=1,
                                      name=f"rinv{hf}")
                nc.vector.reciprocal_approx_fast(out=rinv[:], in_=var[:])
                rs = stat_pool.tile([128, 512], F16, tag="rs16", name=f"rs{hf}")
                nc.scalar.activation(rs[:], rinv[:], AF.Sqrt)
                for k in range(KC):
                    nc.vector.tensor_tensor(h2[:, k, HLF[hf]], y1[:, k, HLF[hf]],
                                            mu[:], op=OP.subtract)
                    nc.vector.tensor_tensor(h2[:, k, HLF[hf]], h2[:, k, HLF[hf]],
                                            rs[:], op=OP.mult)

            outproj_sums(0)
            outproj_sums(1)
            s_op.close()
            ps_c = s_attn.enter_context(
                tc.tile_pool(name="ps_c", bufs=4, space="PSUM"))

            for hf in range(2):
                for n in range(FC):
                    pf = ps_c.tile([128, 512], F32, tag="mm", name=f"pf{n}{hf}")
                    for k in range(KC):
                        nc.tensor.matmul(pf[:], wf1[:, k, n * 128:(n + 1) * 128],
                                         h2[:, k, HLF[hf]],
                                         start=(k == 0), stop=(k == KC - 1))
                    nc.scalar.activation(gl[:, n, :], pf[:], AF.Gelu,
                                         bias=bfc1[:, n:n + 1])
                for m in range(KC):
                    py = ps_c.tile([128, 512], F32, tag="mm", name=f"py{m}{hf}")
                    for kk in range(FC):
                        nc.tensor.matmul(py[:], wf2[:, kk, m * 128:(m + 1) * 128],
                                         gl[:, kk, :],
                                         start=(kk == 0), stop=(kk == FC - 1))
                    yo = yo_pool.tile([128, 512], F16, tag="yo", name=f"yo{m}{hf}")
                    nc.vector.scalar_tensor_tensor(yo[:], py[:], bfc2[:, m:m + 1],
                                                   y1[:, m, HLF[hf]],
                                                   op0=OP.add, op1=OP.add)
                    nc.sync.dma_start(yt_d[m, :, HLF[hf]], yo[:])

        s_attn.close()

    nc.compile()
    return nc


def _get_runner():
    if "runner" in _cached:
        return _cached["runner"]
    import jax
    from jax.sharding import Mesh, PartitionSpec
    from jax.experimental.shard_map import shard_map
    import concourse.mybir as mybir
    from concourse.bass2jax import _bass_exec_p, install_neuronx_cc_hook, partition_id_tensor

    nc = _build()
    install_neuronx_cc_hook()
    partition_name = nc.partition_id_tensor.name if nc.partition_id_tensor else None
    in_names, out_names, out_avals, zero_outs = [], [], [], []
    for alloc in nc.m.functions[0].allocations:
        if not isinstance(alloc, mybir.MemoryLocationSet):
            continue
        name = alloc.memorylocations[0].name
        if alloc.kind == "ExternalInput":
            if name != partition_name:
                in_names.append(name)
        elif alloc.kind == "ExternalOutput":
            out_names.append(name)
            shape = tuple(alloc.tensor_shape)
            dtype = mybir.dt.np(alloc.dtype)
            out_avals.append(jax.core.ShapedArray(shape, dtype))
            zero_outs.append(np.zeros(shape, dtype))
    n_params = len(in_names)
    all_in_names = in_names + out_names + ([partition_name] if partition_name else [])

    def _body(*args):
        operands = list(args)
        if partition_name is not None:
            operands.append(partition_id_tensor())
        outs = _bass_exec_p.bind(
            *operands,
            out_avals=tuple(out_avals),
            in_names=tuple(all_in_names),
            out_names=tuple(out_names),
            lowering_input_output_aliases=(),
            sim_require_finite=False,
            sim_require_nnan=False,
            nc=nc,
        )
        return tuple(outs)

    donate = tuple(range(n_params, n_params + len(out_avals)))
    devices = jax.devices()[:NC]
    mesh = Mesh(np.asarray(devices), ("core",))
    in_specs = (PartitionSpec("core"),) * (n_params + len(out_avals))
    out_specs = (PartitionSpec("core"),) * len(out_names)
    sharded = jax.jit(
        shard_map(_body, mesh=mesh, in_specs=in_specs, out_specs=out_specs, check_rep=False),
        donate_argnums=donate, keep_unused=True,
    )

    runner = {
        "nc": nc, "sharded": sharded, "in_names": in_names,
        "out_names": out_names, "out_avals": out_avals, "zero_outs": zero_outs,
    }
    _cached["runner"] = runner
    return runner


def prep_inputs(x, attn_bias, node_non_padding_mask, in_w, in_b, out_w, out_b,
                ln1_g, ln1_b, fc1_w, fc1_b, fc2_w, fc2_b, ln2_g, ln2_b):
    f16, f32 = np.float16, np.float32
    x = np.asarray(x, f32)
    # LN1 on host (no affine; gamma/beta folded into qkv weights/bias)
    mu = x.mean(-1, keepdims=True)
    var = ((x - mu) ** 2).mean(-1, keepdims=True)
    h1 = (x - mu) / np.sqrt(var + EPS)
    xt = x.transpose(2, 1, 0).reshape(D, N_GRAPH * N_NODE).astype(f16)
    h1t = h1.transpose(2, 1, 0).reshape(D, N_GRAPH * N_NODE).astype(f16)
    xt_pc = [np.ascontiguousarray(xt[:, c * T:(c + 1) * T]).reshape(KC, 128, T) for c in range(NC)]
    h1t_pc = [np.ascontiguousarray(h1t[:, c * T:(c + 1) * T]).reshape(KC, 128, T) for c in range(NC)]
    # exp(bias), transposed per head-graph, key-tiled:
    # ebh[gh, p, kt*512+n] = exp(bias[gh, n, kt*128+p])
    ebt = np.exp(np.asarray(attn_bias, f32)).transpose(0, 2, 1)  # [gh, m, n]
    ebh = np.ascontiguousarray(
        ebt.reshape(N_GRAPH * H, NQT, 128, N_NODE).transpose(0, 2, 1, 3)
    ).reshape(N_GRAPH * H, 128, NQT * N_NODE).astype(f16)
    mask = np.asarray(node_non_padding_mask).astype(f16)

    in_w = np.asarray(in_w, f32)
    in_b = np.asarray(in_b, f32)
    g1 = np.asarray(ln1_g, f32)
    b1 = np.asarray(ln1_b, f32)
    wqkv = in_w * g1[None, :]                  # fold LN1 gamma
    bqkv = in_b + in_w @ b1                    # fold LN1 beta
    scale = HD ** -0.5
    wqkv = wqkv.copy(); bqkv = bqkv.copy()
    wqkv[:D] *= scale                           # fold q scaling
    bqkv[:D] *= scale
    # v bias folded into the out-projection bias: out_w @ bv + out_b
    bv = bqkv[2 * D:]
    bout_full = np.asarray(out_b, f32) + np.asarray(out_w, f32) @ bv
    fc1w = np.asarray(fc1_w, f32)
    fc1b = np.asarray(fc1_b, f32)
    g2 = np.asarray(ln2_g, f32)
    b2 = np.asarray(ln2_b, f32)
    wfc1 = fc1w * g2[None, :]                  # fold LN2 gamma
    bfc1 = fc1b + fc1w @ b2                    # fold LN2 beta

    pm = lambda a, nchunk: np.ascontiguousarray(
        a.reshape(nchunk, 128, a.shape[-1]).transpose(1, 0, 2))  # partition-major
    wqkvT = wqkv.T.astype(f16)                 # [D, 3D]
    shared = {
        "wqk": np.ascontiguousarray(wqkvT[:, :2 * D]).reshape(KC, 128, 2 * D),
        "wv": pm(np.ascontiguousarray(wqkvT[:, 2 * D:]), KC),
        "bqk": np.ascontiguousarray(bqkv[:2 * D].reshape(2 * H, HD).T),
        "wout": pm(np.asarray(out_w, f32).T.astype(f16), KC),
        "bout": np.ascontiguousarray(bout_full.reshape(KC, 128).T),
        "wfc1": pm(wfc1.T.astype(f16), KC),
        "bfc1": np.ascontiguousarray(bfc1.reshape(FC, 128).T),
        "wfc2": pm(np.asarray(fc2_w, f32).T.astype(f16), FC),
        "bfc2": np.ascontiguousarray(np.asarray(fc2_b, f32).reshape(KC, 128).T),
        "ones": np.ones((128, 128), dtype=f16),
    }
    per_core = []
    for c in range(NC):
        m = dict(shared)
        m["xt"] = np.ascontiguousarray(xt_pc[c].transpose(1, 0, 2))
        m["h1t"] = h1t_pc[c]
        m["ebias"] = ebh[G * H * c: G * H * (c + 1)]
        m["maskrow"] = np.ascontiguousarray(mask[G * c: G * (c + 1)]).reshape(1, T)
        per_core.append(m)
    return per_core


def postprocess(outs):
    yt = np.stack([np.asarray(o["yt"], np.float32).reshape(D, T) for o in outs])
    y = yt.reshape(NC, D, G, N_NODE).transpose(3, 0, 2, 1).reshape(N_NODE, N_GRAPH, D)
    return np.ascontiguousarray(y)


def run_per_core(per_core):
    r = _get_runner()
    n = NC
    concat_in = [
        np.concatenate([np.asarray(per_core[c][name]) for c in range(n)], axis=0)
        for name in r["in_names"]
    ]
    concat_zeros = [np.zeros((n * z.shape[0], *z.shape[1:]), z.dtype) for z in r["zero_outs"]]
    out_arrs = r["sharded"](*concat_in, *concat_zeros)
    return [
        {name: np.asarray(out_arrs[i]).reshape(n, *r["out_avals"][i].shape)[c]
         for i, name in enumerate(r["out_names"])}
        for c in range(n)
    ]


def kernel(**inputs):
    per_core = prep_inputs(**inputs)
    outs = run_per_core(per_core)
    return postprocess(outs)


# revision 28
# speedup vs baseline: 1.5172x; 1.0014x over previous
"""Graphormer3D encoder layer on 8 Trainium2 NeuronCores — v3.

Data-parallel over the 16 graphs (2 per core); params replicated.
Feature-major activations (x^T: [feature, token]) fp16, fp32 PSUM.

v3 changes vs v2 (303.7us):
  - h1 = LN1(x) precomputed on host (same class of prep as exp(attn_bias));
    removes LN1 sum matmuls, the ~7us LN1 serial-chain PE gap, and the
    DVE normalize passes. x still shipped for the residual.
  - v computed directly in key-major layout (h1 chunk as stationary,
    wv as moving): kills the per-head PE transposes and v's share of the
    96-row QKV tiles (-20k PE columns). v bias folded into bout on host
    (bout += out_w @ bv), exact.
  - out-proj is hf-major and LN2 runs per-half: LN2 stats/apply for half 0
    overlap out-proj half 1 on the PE; FFN half 0 overlaps LN2 half 1.
    Removes the ~8us LN2 serial-chain PE gap.
  - LN2 rs via a single Rsqrt activation (no Ln->table-load->Exp chain);
    y1^2 computed on ScalarE (Square) instead of DVE.
  - scores/PV interleaved across the two graphs per head (sc g0, sc g1,
    pa g0, pa g1) with psum rings 3/3/2 to cover the exp/mult chain.
  - DMA: h1 chunks on the sync queue and qkv weights on the scalar queue
    issued first (parallel descriptor issue); wfc1/wfc2 issued late from
    the vector queue so they don't steal head bandwidth.
"""
import numpy as np

N_NODE, N_GRAPH, D = 512, 16, 768
H, HD, FFN = 8, 96, 3072
EPS = 1e-5
NC = 8            # cores
G = 2             # graphs per core
T = G * N_NODE    # tokens per core (1024)
KC = D // 128     # 6 feature chunks
FC = FFN // 128   # 24 ffn chunks
NQT = N_NODE // 128  # 4 key tiles per graph
HLF = (slice(0, 512), slice(512, 1024))

_cached = {}


def _build():
    import concourse.bass as bass
    import concourse.mybir as mybir
    import concourse.tile as tile
    import concourse.bacc as bacc
    from contextlib import ExitStack

    F16 = mybir.dt.float16
    F32 = mybir.dt.float32
    AF = mybir.ActivationFunctionType
    OP = mybir.AluOpType

    nc = bacc.Bacc("TRN2", target_bir_lowering=False, debug=False, num_devices=NC)

    di = lambda name, shape, dt: nc.declare_dram_parameter(name, shape, dt, isOutput=False)
    h1_d = di("h1t", [KC, 128, T], F16)
    xt_d = di("xt", [128, KC, T], F16)
    ebias_d = di("ebias", [G * H, 128, NQT * N_NODE], F16)  # exp(bias)^T, tiled
    mask_d = di("maskrow", [1, T], F16)
    wqk_d = di("wqk", [KC, 128, 2 * D], F16)
    wv_d = di("wv", [128, KC, D], F16)
    bqk_d = di("bqk", [HD, 2 * H], F32)
    wout_d = di("wout", [128, KC, D], F16)                 # partition-major
    bout_d = di("bout", [128, KC], F32)
    wfc1_d = di("wfc1", [128, KC, FFN], F16)
    bfc1_d = di("bfc1", [128, FC], F32)
    wfc2_d = di("wfc2", [128, FC, D], F16)
    bfc2_d = di("bfc2", [128, KC], F32)
    ones_d = di("ones", [128, 128], F16)
    yt_d = nc.declare_dram_parameter("yt", [KC, 128, T], F16, isOutput=True)

    with tile.TileContext(nc) as tc, ExitStack() as top:
        # Left-side pools are a LIFO stack: persistent pools first, then h1
        # (freed after attention), then the attention pools. DMA issue order
        # is set separately by the dma_start call order below.
        const = top.enter_context(tc.tile_pool(name="const", bufs=1))
        x_pool = top.enter_context(tc.tile_pool(name="x", bufs=1))
        stat_pool = top.enter_context(tc.tile_pool(name="stat", bufs=2))
        h2_pool = top.enter_context(tc.tile_pool(name="h2", bufs=1))
        y1_pool = top.enter_context(tc.tile_pool(name="y1", bufs=1))
        yo_pool = top.enter_context(tc.tile_pool(name="yo", bufs=2))
        s_attn = ExitStack()
        attn_pool = s_attn.enter_context(tc.tile_pool(name="attn", bufs=1))
        attn_sb = attn_pool.tile([128, KC, T], F16, tag="attn", name="attn_sb")
        s_h1 = ExitStack()
        h1_pool = s_h1.enter_context(tc.tile_pool(name="h1", bufs=1))
        h1 = h1_pool.tile([128, KC, T], F16, tag="h1", name="h1")

        wqk_pool = top.enter_context(tc.tile_pool(name="wqk", bufs=1, side="right"))
        wqk = wqk_pool.tile([128, KC, 2 * D], F16, tag="wqk", name="wqk")
        wout_pool = top.enter_context(tc.tile_pool(name="wout", bufs=1, side="right"))
        wo = wout_pool.tile([128, KC, D], F16, tag="wout", name="wo")
        s_wv = ExitStack()
        wv_pool = s_wv.enter_context(tc.tile_pool(name="wv", bufs=1, side="right"))
        wv = wv_pool.tile([128, KC, D], F16, tag="wv", name="wv")
        x_big = x_pool.tile([128, KC, T], F16, tag="x", name="x_big")

        # ---------------- DMA: critical path first ----------------
        # h1 chunks on sync queue; qkv weights on scalar queue (parallel
        # descriptor issue; first QKV matmul gated only on h1[0]+wqk[0]).
        for k in range(KC):
            nc.sync.dma_start(h1[:, k, :], h1_d[k])
            nc.scalar.dma_start(wqk[:, k, :], wqk_d[k])
        nc.scalar.dma_start(wv[:], wv_d[:])

        def load_const(name, dram, shape, dt, eng=nc.gpsimd):
            t = const.tile(shape, dt, tag=name, name=name)
            eng.dma_start(t[:], dram[:])
            return t

        bqk = load_const("bqk", bqk_d, [HD, 2 * H], F32)
        mask_sb = load_const("mask", mask_d, [1, T], F16)
        ones = load_const("ones", ones_d, [128, 128], F16)
        bout = load_const("bout", bout_d, [128, KC], F32)
        bfc1 = load_const("bfc1", bfc1_d, [128, FC], F32)
        bfc2 = load_const("bfc2", bfc2_d, [128, KC], F32)
        eps_sb = const.tile([128, 1], F32, tag="eps", name="eps")
        nc.vector.memset(eps_sb[:], EPS)

        # remaining weights on scalar queue, after the qkv-critical ones
        nc.scalar.dma_start(wo[:], wout_d[:])
        nc.scalar.dma_start(x_big[:], xt_d[:])

        # ---------------- attention phase ----------------
        s_ap = ExitStack()
        vx_pool = s_ap.enter_context(tc.tile_pool(name="vx", bufs=1))
        vx = vx_pool.tile([128, G * H, NQT, HD + 1], F16, tag="vx", name="vx")
        nc.vector.memset(vx[:, :, :, HD:HD + 1], 1.0)
        qkvt_pool = s_ap.enter_context(tc.tile_pool(name="qkvt", bufs=2))
        eb_pool = s_ap.enter_context(tc.tile_pool(name="eb", bufs=4))
        p_pool = s_ap.enter_context(tc.tile_pool(name="p", bufs=3))
        small_pool = s_ap.enter_context(tc.tile_pool(name="small", bufs=2))
        ps_pa = s_ap.enter_context(tc.tile_pool(name="ps_pa", bufs=2, space="PSUM"))
        ps_sc = s_ap.enter_context(tc.tile_pool(name="ps_sc", bufs=3, space="PSUM"))
        ps_qk = s_ap.enter_context(tc.tile_pool(name="ps_qk", bufs=3, space="PSUM"))

        def qkv_head(th, tag):
            """th in 0..15: q heads 0-7 then k heads 0-7."""
            t = qkvt_pool.tile([HD, T], F16, tag=tag, name=f"{tag}{th}")
            for hf in range(2):
                ps = ps_qk.tile([HD, 512], F32, tag="qk", name=f"qk{th}{hf}")
                for k in range(KC):
                    nc.tensor.matmul(
                        ps[:], wqk[:, k, th * HD:(th + 1) * HD],
                        h1[:, k, HLF[hf]], start=(k == 0), stop=(k == KC - 1))
                if th < H:       # q evict on DVE to balance engines
                    nc.vector.tensor_scalar_add(t[:, HLF[hf]], ps[:],
                                                bqk[:, th:th + 1])
                else:            # k evict on ScalarE
                    nc.scalar.activation(t[:, HLF[hf]], ps[:], AF.Identity,
                                         bias=bqk[:, th:th + 1])
            return t

        def v_direct(g):
            """v for all 8 heads of graph g, key-major, into vx.
            out[tok128, hd] = h1_chunk^T @ wv_chunk; borrow the sc psum ring.
            v bias is folded into bout on host."""
            base = g * N_NODE
            for kt in range(NQT):
                tok = slice(base + kt * 128, base + (kt + 1) * 128)
                for half in range(2):
                    vp = ps_sc.tile([128, 512], F32, tag="sc", name=f"v{g}{kt}{half}")
                    cols = slice(half * 384, (half + 1) * 384)
                    for k in range(KC):
                        nc.tensor.matmul(vp[:, 0:384], h1[:, k, tok],
                                         wv[:, k, cols],
                                         start=(k == 0), stop=(k == KC - 1))
                    h0 = half * 4
                    nc.scalar.activation(
                        vx[:, g * H + h0:g * H + h0 + 4, kt, 0:HD],
                        vp[:, 0:384], AF.Copy)

        eb_tiles = {}

        def eb_prefetch(hh):
            for g in range(G):
                gh = g * H + hh
                eb = eb_pool.tile([128, NQT, N_NODE], F16, tag="eb", name=f"eb{gh}")
                nc.sync.dma_start(eb[:], ebias_d[gh])
                eb_tiles[gh] = eb

        def attn_scores(g, hh, q_t, k_t):
            gh = g * H + hh
            base = g * N_NODE
            eb = eb_tiles.pop(gh)
            p = p_pool.tile([128, NQT, N_NODE], F16, tag="p", name=f"p{gh}")
            for kt in range(NQT):
                sc = ps_sc.tile([128, N_NODE], F32, tag="sc", name=f"sc{gh}_{kt}")
                nc.tensor.matmul(sc[:],
                                 k_t[:, base + kt * 128: base + (kt + 1) * 128],
                                 q_t[:, base: base + N_NODE],
                                 start=True, stop=True)
                nc.scalar.activation(p[:, kt, :], sc[:], AF.Exp)
                nc.vector.tensor_tensor(p[:, kt, :], p[:, kt, :],
                                        eb[:, kt, :], op=OP.mult)
            return p

        def attn_pv(g, hh, p):
            gh = g * H + hh
            base = g * N_NODE
            pa = ps_pa.tile([HD + 1, N_NODE], F32, tag="pa", name=f"pa{gh}")
            for kt in range(NQT):
                nc.tensor.matmul(pa[:], vx[:, gh, kt, :], p[:, kt, :],
                                 start=(kt == 0), stop=(kt == NQT - 1))
            s_sb = small_pool.tile([1, N_NODE], F32, tag="s", name=f"s{gh}")
            nc.scalar.activation(s_sb[:], pa[HD:HD + 1, :], AF.Copy)
            r = small_pool.tile([1, N_NODE], F32, tag="r", name=f"r{gh}")
            nc.vector.reciprocal_approx_fast(out=r[:], in_=s_sb[:])
            # fold the padding mask into r: zeroes padded query tokens
            # before the (linear) out-projection — exact.
            nc.vector.tensor_tensor(r[:], r[:], mask_sb[:, base: base + N_NODE],
                                    op=OP.mult)
            rb = small_pool.tile([HD, N_NODE], F32, tag="rb", name=f"rb{gh}")
            nc.gpsimd.partition_broadcast(rb[:], r[:])
            # normalized evict into packed attn chunks; pieces limited by dst
            # chunk boundary and PSUM quadrant reads (src offset != 0 allows
            # at most 32 partitions per access)
            R0 = hh * HD
            allowed = lambda p_: 128 if p_ == 0 else (64 if p_ == 64 else 32)
            off = 0
            while off < HD:
                c, rr = (R0 + off) // 128, (R0 + off) % 128
                ln = min(HD - off, 128 - rr, allowed(off), allowed(rr))
                nc.vector.tensor_tensor(
                    attn_sb[rr:rr + ln, c, base: base + N_NODE],
                    pa[off:off + ln, :], rb[off:off + ln, :], op=OP.mult)
                off += ln

        eb_prefetch(0)
        eb_prefetch(1)
        for hh in range(H):
            if hh + 2 < H:
                eb_prefetch(hh + 2)
            q_t = qkv_head(hh, "q")
            k_t = qkv_head(H + hh, "k")
            if hh == 0:
                v_direct(0)
                v_direct(1)
                s_wv.close()
            if hh == 1:
                # wfc1 issued from vector queue now: doesn't steal head
                # bandwidth, arrives well before the FFN needs it.
                wfc1_pool = top.enter_context(
                    tc.tile_pool(name="wfc1", bufs=1, side="right"))
                wf1 = wfc1_pool.tile([128, KC, FFN], F16, tag="wfc1", name="wf1")
                nc.gpsimd.dma_start(wf1[:], wfc1_d[:])
            p0 = attn_scores(0, hh, q_t, k_t)
            p1 = attn_scores(1, hh, q_t, k_t)
            attn_pv(0, hh, p0)
            attn_pv(1, hh, p1)

        s_ap.close()
        s_h1.close()

        wfc2_pool = top.enter_context(
            tc.tile_pool(name="wfc2", bufs=1, side="right"))
        wf2 = wfc2_pool.tile([128, FC, D], F16, tag="wfc2", name="wf2")
        nc.gpsimd.dma_start(wf2[:], wfc2_d[:])

        # ---------------- out-proj + per-half LN2 + FFN ----------------
        # PE stream order: op(0), sums(0), op(1), sums(1), fc1/fc2(0),
        # fc1/fc2(1) — each LN2 serial chain is covered by the following
        # block's matmuls. Out-proj uses 6 parallel psum banks per half with
        # chunks 4,5 accumulated last (they are written by the last heads).
        ps_s = s_attn.enter_context(tc.tile_pool(name="ps_s", bufs=2, space="PSUM"))
        s_op = ExitStack()
        ps_op = s_op.enter_context(tc.tile_pool(name="ps_op", bufs=6, space="PSUM"))

        y1 = y1_pool.tile([128, KC, T], F16, tag="y1", name="y1")
        h2 = h2_pool.tile([128, KC, T], F16, tag="h2", name="h2")

        with tc.tile_pool(name="sq", bufs=1) as sq_pool, \
             tc.tile_pool(name="gelu", bufs=1) as gelu_pool:
            sq = sq_pool.tile([128, KC, T], F16, tag="sq", name="sq")
            gl = gelu_pool.tile([128, FC, 512], F16, tag="gelu", name="gl")
            mus, rss = {}, {}

            def outproj_sums(hf):
                pos = [ps_op.tile([128, 512], F32, tag="po", name=f"po{m}{hf}")
                       for m in range(KC)]
                C_ORDER = [0, 1, 2, 3, 4, 5]
                for ci, c in enumerate(C_ORDER):
                    for m in range(KC):
                        nc.tensor.matmul(pos[m][:], wo[:, c, m * 128:(m + 1) * 128],
                                         attn_sb[:, c, HLF[hf]],
                                         start=(ci == 0), stop=(ci == KC - 1))
                for m in range(KC):
                    # mask is folded into the softmax reciprocal r, so the
                    # evict is one fused op: y1 = (po + bout) + x
                    nc.vector.scalar_tensor_tensor(y1[:, m, HLF[hf]], pos[m][:],
                                                   bout[:, m:m + 1],
                                                   x_big[:, m, HLF[hf]],
                                                   op0=OP.add, op1=OP.add)
                    nc.scalar.activation(sq[:, m, HLF[hf]], y1[:, m, HLF[hf]],
                                         AF.Square)
                # LN2 stats for this half
                pss = ps_s.tile([128, 512], F32, tag="sum", name=f"lns{hf}")
                for k in range(KC):
                    nc.tensor.matmul(pss[:], ones[:], y1[:, k, HLF[hf]],
                                     start=(k == 0), stop=(k == KC - 1))
                psq = ps_s.tile([128, 512], F32, tag="sum", name=f"lnq{hf}")
                for k in range(KC):
                    nc.tensor.matmul(psq[:], ones[:], sq[:, k, HLF[hf]],
                                     start=(k == 0), stop=(k == KC - 1))
                mu = stat_pool.tile([128, 512], F16, tag="mu16", name=f"mu{hf}")
                nc.vector.tensor_scalar_mul(mu[:], pss[:], 1.0 / D)
                m2 = stat_pool.tile([128, 512], F32, tag="m2", bufs=1, name=f"m2{hf}")
                nc.vector.tensor_tensor(m2[:], mu[:], mu[:], op=OP.mult)
                nc.vector.tensor_scalar_sub(m2[:], m2[:], EPS)   # var+eps below
                var = stat_pool.tile([128, 512], F32, tag="var", bufs=1, name=f"var{hf}")
                nc.vector.scalar_tensor_tensor(var[:], psq[:], 1.0 / D,
                                               m2[:], op0=OP.mult, op1=OP.subtract)
                rinv = stat_pool.tile([128, 512], F32, tag="rinv", bufs=1,
                                      name=f"rinv{hf}")
                nc.vector.reciprocal_approx_fast(out=rinv[:], in_=var[:])
                rs = stat_pool.tile([128, 512], F16, tag="rs16", name=f"rs{hf}")
                nc.scalar.activation(rs[:], rinv[:], AF.Sqrt)
                for k in range(KC):
                    nc.vector.tensor_tensor(h2[:, k, HLF[hf]], y1[:, k, HLF[hf]],
                                            mu[:], op=OP.subtract)
                    nc.vector.tensor_tensor(h2[:, k, HLF[hf]], h2[:, k, HLF[hf]],
                                            rs[:], op=OP.mult)

            outproj_sums(0)
            outproj_sums(1)
            s_op.close()
            ps_c = s_attn.enter_context(
                tc.tile_pool(name="ps_c", bufs=4, space="PSUM"))

            for hf in range(2):
                for n in range(FC):
                    pf = ps_c.tile([128, 512], F32, tag="mm", name=f"pf{n}{hf}")
                    for k in range(KC):
                        nc.tensor.matmul(pf[:], wf1[:, k, n * 128:(n + 1) * 128],
                                         h2[:, k, HLF[hf]],
                                         start=(k == 0), stop=(k == KC - 1))
                    nc.scalar.activation(gl[:, n, :], pf[:], AF.Gelu,
                                         bias=bfc1[:, n:n + 1])
                for m in range(KC):
                    py = ps_c.tile([128, 512], F32, tag="mm", name=f"py{m}{hf}")
                    for kk in range(FC):
                        nc.tensor.matmul(py[:], wf2[:, kk, m * 128:(m + 1) * 128],
                                         gl[:, kk, :],
                                         start=(kk == 0), stop=(kk == FC - 1))
                    yo = yo_pool.tile([128, 512], F16, tag="yo", name=f"yo{m}{hf}")
                    nc.vector.scalar_tensor_tensor(yo[:], py[:], bfc2[:, m:m + 1],
                                                   y1[:, m, HLF[hf]],
                                                   op0=OP.add, op1=OP.add)
                    nc.sync.dma_start(yt_d[m, :, HLF[hf]], yo[:])

        s_attn.close()

    nc.compile()
    return nc


def _get_runner():
    if "runner" in _cached:
        return _cached["runner"]
    import jax
    from jax.sharding import Mesh, PartitionSpec
    from jax.experimental.shard_map import shard_map
    import concourse.mybir as mybir
    from concourse.bass2jax import _bass_exec_p, install_neuronx_cc_hook, partition_id_tensor

    nc = _build()
    install_neuronx_cc_hook()
    partition_name = nc.partition_id_tensor.name if nc.partition_id_tensor else None
    in_names, out_names, out_avals, zero_outs = [], [], [], []
    for alloc in nc.m.functions[0].allocations:
        if not isinstance(alloc, mybir.MemoryLocationSet):
            continue
        name = alloc.memorylocations[0].name
        if alloc.kind == "ExternalInput":
            if name != partition_name:
                in_names.append(name)
        elif alloc.kind == "ExternalOutput":
            out_names.append(name)
            shape = tuple(alloc.tensor_shape)
            dtype = mybir.dt.np(alloc.dtype)
            out_avals.append(jax.core.ShapedArray(shape, dtype))
            zero_outs.append(np.zeros(shape, dtype))
    n_params = len(in_names)
    all_in_names = in_names + out_names + ([partition_name] if partition_name else [])

    def _body(*args):
        operands = list(args)
        if partition_name is not None:
            operands.append(partition_id_tensor())
        outs = _bass_exec_p.bind(
            *operands,
            out_avals=tuple(out_avals),
            in_names=tuple(all_in_names),
            out_names=tuple(out_names),
            lowering_input_output_aliases=(),
            sim_require_finite=False,
            sim_require_nnan=False,
            nc=nc,
        )
        return tuple(outs)

    donate = tuple(range(n_params, n_params + len(out_avals)))
    devices = jax.devices()[:NC]
    mesh = Mesh(np.asarray(devices), ("core",))
    in_specs = (PartitionSpec("core"),) * (n_params + len(out_avals))
    out_specs = (PartitionSpec("core"),) * len(out_names)
    sharded = jax.jit(
        shard_map(_body, mesh=mesh, in_specs=in_specs, out_specs=out_specs, check_rep=False),
        donate_argnums=donate, keep_unused=True,
    )

    runner = {
        "nc": nc, "sharded": sharded, "in_names": in_names,
        "out_names": out_names, "out_avals": out_avals, "zero_outs": zero_outs,
    }
    _cached["runner"] = runner
    return runner


def prep_inputs(x, attn_bias, node_non_padding_mask, in_w, in_b, out_w, out_b,
                ln1_g, ln1_b, fc1_w, fc1_b, fc2_w, fc2_b, ln2_g, ln2_b):
    f16, f32 = np.float16, np.float32
    x = np.asarray(x, f32)
    # LN1 on host (no affine; gamma/beta folded into qkv weights/bias)
    mu = x.mean(-1, keepdims=True)
    var = ((x - mu) ** 2).mean(-1, keepdims=True)
    h1 = (x - mu) / np.sqrt(var + EPS)
    xt = x.transpose(2, 1, 0).reshape(D, N_GRAPH * N_NODE).astype(f16)
    h1t = h1.transpose(2, 1, 0).reshape(D, N_GRAPH * N_NODE).astype(f16)
    xt_pc = [np.ascontiguousarray(xt[:, c * T:(c + 1) * T]).reshape(KC, 128, T) for c in range(NC)]
    h1t_pc = [np.ascontiguousarray(h1t[:, c * T:(c + 1) * T]).reshape(KC, 128, T) for c in range(NC)]
    # exp(bias), transposed per head-graph, key-tiled:
    # ebh[gh, p, kt*512+n] = exp(bias[gh, n, kt*128+p])
    ebt = np.exp(np.asarray(attn_bias, f32)).transpose(0, 2, 1)  # [gh, m, n]
    ebh = np.ascontiguousarray(
        ebt.reshape(N_GRAPH * H, NQT, 128, N_NODE).transpose(0, 2, 1, 3)
    ).reshape(N_GRAPH * H, 128, NQT * N_NODE).astype(f16)
    mask = np.asarray(node_non_padding_mask).astype(f16)

    in_w = np.asarray(in_w, f32)
    in_b = np.asarray(in_b, f32)
    g1 = np.asarray(ln1_g, f32)
    b1 = np.asarray(ln1_b, f32)
    wqkv = in_w * g1[None, :]                  # fold LN1 gamma
    bqkv = in_b + in_w @ b1                    # fold LN1 beta
    scale = HD ** -0.5
    wqkv = wqkv.copy(); bqkv = bqkv.copy()
    wqkv[:D] *= scale                           # fold q scaling
    bqkv[:D] *= scale
    # v bias folded into the out-projection bias: out_w @ bv + out_b
    bv = bqkv[2 * D:]
    bout_full = np.asarray(out_b, f32) + np.asarray(out_w, f32) @ bv
    fc1w = np.asarray(fc1_w, f32)
    fc1b = np.asarray(fc1_b, f32)
    g2 = np.asarray(ln2_g, f32)
    b2 = np.asarray(ln2_b, f32)
    wfc1 = fc1w * g2[None, :]                  # fold LN2 gamma
    bfc1 = fc1b + fc1w @ b2                    # fold LN2 beta

    pm = lambda a, nchunk: np.ascontiguousarray(
        a.reshape(nchunk, 128, a.shape[-1]).transpose(1, 0, 2))  # partition-major
    wqkvT = wqkv.T.astype(f16)                 # [D, 3D]
    shared = {
        "wqk": np.ascontiguousarray(wqkvT[:, :2 * D]).reshape(KC, 128, 2 * D),
        "wv": pm(np.ascontiguousarray(wqkvT[:, 2 * D:]), KC),
        "bqk": np.ascontiguousarray(bqkv[:2 * D].reshape(2 * H, HD).T),
        "wout": pm(np.asarray(out_w, f32).T.astype(f16), KC),
        "bout": np.ascontiguousarray(bout_full.reshape(KC, 128).T),
        "wfc1": pm(wfc1.T.astype(f16), KC),
        "bfc1": np.ascontiguousarray(bfc1.reshape(FC, 128).T),
        "wfc2": pm(np.asarray(fc2_w, f32).T.astype(f16), FC),
        "bfc2": np.ascontiguousarray(np.asarray(fc2_b, f32).reshape(KC, 128).T),
        "ones": np.ones((128, 128), dtype=f16),
    }
    per_core = []
    for c in range(NC):
        m = dict(shared)
        m["xt"] = np.ascontiguousarray(xt_pc[c].transpose(1, 0, 2))
        m["h1t"] = h1t_pc[c]
        m["ebias"] = ebh[G * H * c: G * H * (c + 1)]
        m["maskrow"] = np.ascontiguousarray(mask[G * c: G * (c + 1)]).reshape(1, T)
        per_core.append(m)
    return per_core


def postprocess(outs):
    yt = np.stack([np.asarray(o["yt"], np.float32).reshape(D, T) for o in outs])
    y = yt.reshape(NC, D, G, N_NODE).transpose(3, 0, 2, 1).reshape(N_NODE, N_GRAPH, D)
    return np.ascontiguousarray(y)


def run_per_core(per_core):
    r = _get_runner()
    n = NC
    concat_in = [
        np.concatenate([np.asarray(per_core[c][name]) for c in range(n)], axis=0)
        for name in r["in_names"]
    ]
    concat_zeros = [np.zeros((n * z.shape[0], *z.shape[1:]), z.dtype) for z in r["zero_outs"]]
    out_arrs = r["sharded"](*concat_in, *concat_zeros)
    return [
        {name: np.asarray(out_arrs[i]).reshape(n, *r["out_avals"][i].shape)[c]
         for i, name in enumerate(r["out_names"])}
        for c in range(n)
    ]


def kernel(**inputs):
    per_core = prep_inputs(**inputs)
    outs = run_per_core(per_core)
    return postprocess(outs)


# revision 29
# speedup vs baseline: 1.5574x; 1.0265x over previous
"""Graphormer3D encoder layer on 8 Trainium2 NeuronCores — v3.

Data-parallel over the 16 graphs (2 per core); params replicated.
Feature-major activations (x^T: [feature, token]) fp16, fp32 PSUM.

v3 changes vs v2 (303.7us):
  - h1 = LN1(x) precomputed on host (same class of prep as exp(attn_bias));
    removes LN1 sum matmuls, the ~7us LN1 serial-chain PE gap, and the
    DVE normalize passes. x still shipped for the residual.
  - v computed directly in key-major layout (h1 chunk as stationary,
    wv as moving): kills the per-head PE transposes and v's share of the
    96-row QKV tiles (-20k PE columns). v bias folded into bout on host
    (bout += out_w @ bv), exact.
  - out-proj is hf-major and LN2 runs per-half: LN2 stats/apply for half 0
    overlap out-proj half 1 on the PE; FFN half 0 overlaps LN2 half 1.
    Removes the ~8us LN2 serial-chain PE gap.
  - LN2 rs via a single Rsqrt activation (no Ln->table-load->Exp chain);
    y1^2 computed on ScalarE (Square) instead of DVE.
  - scores/PV interleaved across the two graphs per head (sc g0, sc g1,
    pa g0, pa g1) with psum rings 3/3/2 to cover the exp/mult chain.
  - DMA: h1 chunks on the sync queue and qkv weights on the scalar queue
    issued first (parallel descriptor issue); wfc1/wfc2 issued late from
    the vector queue so they don't steal head bandwidth.
"""
import numpy as np

N_NODE, N_GRAPH, D = 512, 16, 768
H, HD, FFN = 8, 96, 3072
EPS = 1e-5
NC = 8            # cores
G = 2             # graphs per core
T = G * N_NODE    # tokens per core (1024)
KC = D // 128     # 6 feature chunks
FC = FFN // 128   # 24 ffn chunks
NQT = N_NODE // 128  # 4 key tiles per graph
HLF = (slice(0, 512), slice(512, 1024))

_cached = {}


def _build():
    import concourse.bass as bass
    import concourse.mybir as mybir
    import concourse.tile as tile
    import concourse.bacc as bacc
    from contextlib import ExitStack

    F16 = mybir.dt.float16
    F32 = mybir.dt.float32
    AF = mybir.ActivationFunctionType
    OP = mybir.AluOpType

    nc = bacc.Bacc("TRN2", target_bir_lowering=False, debug=False, num_devices=NC)

    di = lambda name, shape, dt: nc.declare_dram_parameter(name, shape, dt, isOutput=False)
    h1_d = di("h1t", [KC, 128, T], F16)
    xt_d = di("xt", [128, KC, T], F16)
    ebias_d = di("ebias", [G * H, 128, NQT * N_NODE], F16)  # exp(bias)^T, tiled
    mask_d = di("maskrow", [1, T], F16)
    wqk_d = di("wqk", [KC, 128, 2 * D], F16)
    wv_d = di("wv", [128, KC, D], F16)
    bqk_d = di("bqk", [HD, 2 * H], F32)
    wout_d = di("wout", [128, KC, D], F16)                 # partition-major
    bout_d = di("bout", [128, KC], F32)
    wfc1_d = di("wfc1", [128, KC, FFN], F16)
    bfc1_d = di("bfc1", [128, FC], F32)
    wfc2_d = di("wfc2", [128, FC, D], F16)
    bfc2_d = di("bfc2", [128, KC], F32)
    ones_d = di("ones", [128, 128], F16)
    yt_d = nc.declare_dram_parameter("yt", [KC, 128, T], F16, isOutput=True)

    with tile.TileContext(nc) as tc, ExitStack() as top:
        # Left-side pools are a LIFO stack: persistent pools first, then h1
        # (freed after attention), then the attention pools. DMA issue order
        # is set separately by the dma_start call order below.
        const = top.enter_context(tc.tile_pool(name="const", bufs=1))
        x_pool = top.enter_context(tc.tile_pool(name="x", bufs=1))
        stat_pool = top.enter_context(tc.tile_pool(name="stat", bufs=2))
        h2_pool = top.enter_context(tc.tile_pool(name="h2", bufs=1))
        y1_pool = top.enter_context(tc.tile_pool(name="y1", bufs=1))
        yo_pool = top.enter_context(tc.tile_pool(name="yo", bufs=2))
        s_attn = ExitStack()
        attn_pool = s_attn.enter_context(tc.tile_pool(name="attn", bufs=1))
        attn_sb = attn_pool.tile([128, KC, T], F16, tag="attn", name="attn_sb")
        s_h1 = ExitStack()
        h1_pool = s_h1.enter_context(tc.tile_pool(name="h1", bufs=1))
        h1 = h1_pool.tile([128, KC, T], F16, tag="h1", name="h1")

        wqk_pool = top.enter_context(tc.tile_pool(name="wqk", bufs=1, side="right"))
        wqk = wqk_pool.tile([128, KC, 2 * D], F16, tag="wqk", name="wqk")
        wout_pool = top.enter_context(tc.tile_pool(name="wout", bufs=1, side="right"))
        wo = wout_pool.tile([128, KC, D], F16, tag="wout", name="wo")
        s_wv = ExitStack()
        wv_pool = s_wv.enter_context(tc.tile_pool(name="wv", bufs=1, side="right"))
        wv = wv_pool.tile([128, KC, D], F16, tag="wv", name="wv")
        x_big = x_pool.tile([128, KC, T], F16, tag="x", name="x_big")

        # ---------------- DMA: critical path first ----------------
        # h1 chunks on sync queue; qkv weights on scalar queue (parallel
        # descriptor issue; first QKV matmul gated only on h1[0]+wqk[0]).
        for k in range(KC):
            nc.sync.dma_start(h1[:, k, :], h1_d[k])
            nc.scalar.dma_start(wqk[:, k, :], wqk_d[k])
        nc.scalar.dma_start(wv[:], wv_d[:])

        def load_const(name, dram, shape, dt, eng=nc.gpsimd):
            t = const.tile(shape, dt, tag=name, name=name)
            eng.dma_start(t[:], dram[:])
            return t

        bqk = load_const("bqk", bqk_d, [HD, 2 * H], F32)
        mask_sb = load_const("mask", mask_d, [1, T], F16)
        ones = load_const("ones", ones_d, [128, 128], F16)
        bout = load_const("bout", bout_d, [128, KC], F32)
        bfc1 = load_const("bfc1", bfc1_d, [128, FC], F32)
        bfc2 = load_const("bfc2", bfc2_d, [128, KC], F32)
        eps_sb = const.tile([128, 1], F32, tag="eps", name="eps")
        nc.vector.memset(eps_sb[:], EPS)

        # remaining weights on scalar queue, after the qkv-critical ones
        nc.scalar.dma_start(wo[:], wout_d[:])
        nc.scalar.dma_start(x_big[:], xt_d[:])

        # ---------------- attention phase ----------------
        s_ap = ExitStack()
        vx_pool = s_ap.enter_context(tc.tile_pool(name="vx", bufs=1))
        vx = vx_pool.tile([128, G * H, NQT, HD + 1], F16, tag="vx", name="vx")
        nc.vector.memset(vx[:, :, :, HD:HD + 1], 1.0)
        qkvt_pool = s_ap.enter_context(tc.tile_pool(name="qkvt", bufs=2))
        eb_pool = s_ap.enter_context(tc.tile_pool(name="eb", bufs=6))
        p_pool = s_ap.enter_context(tc.tile_pool(name="p", bufs=3))
        small_pool = s_ap.enter_context(tc.tile_pool(name="small", bufs=2))
        ps_pa = s_ap.enter_context(tc.tile_pool(name="ps_pa", bufs=2, space="PSUM"))
        ps_sc = s_ap.enter_context(tc.tile_pool(name="ps_sc", bufs=3, space="PSUM"))
        ps_qk = s_ap.enter_context(tc.tile_pool(name="ps_qk", bufs=3, space="PSUM"))

        def qkv_head(th, tag):
            """th in 0..15: q heads 0-7 then k heads 0-7."""
            t = qkvt_pool.tile([HD, T], F16, tag=tag, name=f"{tag}{th}")
            for hf in range(2):
                ps = ps_qk.tile([HD, 512], F32, tag="qk", name=f"qk{th}{hf}")
                for k in range(KC):
                    nc.tensor.matmul(
                        ps[:], wqk[:, k, th * HD:(th + 1) * HD],
                        h1[:, k, HLF[hf]], start=(k == 0), stop=(k == KC - 1))
                if th < H:       # q evict on DVE to balance engines
                    nc.vector.tensor_scalar_add(t[:, HLF[hf]], ps[:],
                                                bqk[:, th:th + 1])
                else:            # k evict on ScalarE
                    nc.scalar.activation(t[:, HLF[hf]], ps[:], AF.Identity,
                                         bias=bqk[:, th:th + 1])
            return t

        def v_direct(g):
            """v for all 8 heads of graph g, key-major, into vx.
            out[tok128, hd] = h1_chunk^T @ wv_chunk; borrow the sc psum ring.
            v bias is folded into bout on host."""
            base = g * N_NODE
            for kt in range(NQT):
                tok = slice(base + kt * 128, base + (kt + 1) * 128)
                for half in range(2):
                    vp = ps_sc.tile([128, 512], F32, tag="sc", name=f"v{g}{kt}{half}")
                    cols = slice(half * 384, (half + 1) * 384)
                    for k in range(KC):
                        nc.tensor.matmul(vp[:, 0:384], h1[:, k, tok],
                                         wv[:, k, cols],
                                         start=(k == 0), stop=(k == KC - 1))
                    h0 = half * 4
                    nc.scalar.activation(
                        vx[:, g * H + h0:g * H + h0 + 4, kt, 0:HD],
                        vp[:, 0:384], AF.Copy)

        eb_tiles = {}

        def eb_prefetch(hh):
            for g in range(G):
                gh = g * H + hh
                eb = eb_pool.tile([128, NQT, N_NODE], F16, tag="eb", name=f"eb{gh}")
                nc.sync.dma_start(eb[:], ebias_d[gh])
                eb_tiles[gh] = eb

        def attn_scores(g, hh, q_t, k_t):
            gh = g * H + hh
            base = g * N_NODE
            eb = eb_tiles.pop(gh)
            p = p_pool.tile([128, NQT, N_NODE], F16, tag="p", name=f"p{gh}")
            for kt in range(NQT):
                sc = ps_sc.tile([128, N_NODE], F32, tag="sc", name=f"sc{gh}_{kt}")
                nc.tensor.matmul(sc[:],
                                 k_t[:, base + kt * 128: base + (kt + 1) * 128],
                                 q_t[:, base: base + N_NODE],
                                 start=True, stop=True)
                nc.scalar.activation(p[:, kt, :], sc[:], AF.Exp)
                nc.vector.tensor_tensor(p[:, kt, :], p[:, kt, :],
                                        eb[:, kt, :], op=OP.mult)
            return p

        def attn_pv(g, hh, p):
            gh = g * H + hh
            base = g * N_NODE
            pa = ps_pa.tile([HD + 1, N_NODE], F32, tag="pa", name=f"pa{gh}")
            for kt in range(NQT):
                nc.tensor.matmul(pa[:], vx[:, gh, kt, :], p[:, kt, :],
                                 start=(kt == 0), stop=(kt == NQT - 1))
            s_sb = small_pool.tile([1, N_NODE], F32, tag="s", name=f"s{gh}")
            nc.scalar.activation(s_sb[:], pa[HD:HD + 1, :], AF.Copy)
            r = small_pool.tile([1, N_NODE], F32, tag="r", name=f"r{gh}")
            nc.vector.reciprocal_approx_fast(out=r[:], in_=s_sb[:])
            # fold the padding mask into r: zeroes padded query tokens
            # before the (linear) out-projection — exact.
            nc.vector.tensor_tensor(r[:], r[:], mask_sb[:, base: base + N_NODE],
                                    op=OP.mult)
            rb = small_pool.tile([HD, N_NODE], F32, tag="rb", name=f"rb{gh}")
            nc.gpsimd.partition_broadcast(rb[:], r[:])
            # normalized evict into packed attn chunks; pieces limited by dst
            # chunk boundary and PSUM quadrant reads (src offset != 0 allows
            # at most 32 partitions per access)
            R0 = hh * HD
            allowed = lambda p_: 128 if p_ == 0 else (64 if p_ == 64 else 32)
            off = 0
            while off < HD:
                c, rr = (R0 + off) // 128, (R0 + off) % 128
                ln = min(HD - off, 128 - rr, allowed(off), allowed(rr))
                nc.vector.tensor_tensor(
                    attn_sb[rr:rr + ln, c, base: base + N_NODE],
                    pa[off:off + ln, :], rb[off:off + ln, :], op=OP.mult)
                off += ln

        eb_prefetch(0)
        eb_prefetch(1)
        eb_prefetch(2)
        for hh in range(H):
            if hh + 3 < H:
                eb_prefetch(hh + 3)
            q_t = qkv_head(hh, "q")
            k_t = qkv_head(H + hh, "k")
            if hh == 0:
                v_direct(0)
                v_direct(1)
                s_wv.close()
            if hh == 3:
                # wfc1 issued mid-attention from the gpsimd queue: late enough
                # not to starve the eb stream, early enough for the FFN.
                wfc1_pool = top.enter_context(
                    tc.tile_pool(name="wfc1", bufs=1, side="right"))
                wf1 = wfc1_pool.tile([128, KC, FFN], F16, tag="wfc1", name="wf1")
                nc.gpsimd.dma_start(wf1[:], wfc1_d[:])
            p0 = attn_scores(0, hh, q_t, k_t)
            p1 = attn_scores(1, hh, q_t, k_t)
            attn_pv(0, hh, p0)
            attn_pv(1, hh, p1)

        s_ap.close()
        s_h1.close()

        wfc2_pool = top.enter_context(
            tc.tile_pool(name="wfc2", bufs=1, side="right"))
        wf2 = wfc2_pool.tile([128, FC, D], F16, tag="wfc2", name="wf2")
        nc.gpsimd.dma_start(wf2[:], wfc2_d[:])

        # ---------------- out-proj + per-half LN2 + FFN ----------------
        # PE stream order: op(0), sums(0), op(1), sums(1), fc1/fc2(0),
        # fc1/fc2(1) — each LN2 serial chain is covered by the following
        # block's matmuls. Out-proj uses 6 parallel psum banks per half with
        # chunks 4,5 accumulated last (they are written by the last heads).
        ps_s = s_attn.enter_context(tc.tile_pool(name="ps_s", bufs=2, space="PSUM"))
        s_op = ExitStack()
        ps_op = s_op.enter_context(tc.tile_pool(name="ps_op", bufs=6, space="PSUM"))

        y1 = y1_pool.tile([128, KC, T], F16, tag="y1", name="y1")
        h2 = h2_pool.tile([128, KC, T], F16, tag="h2", name="h2")

        with tc.tile_pool(name="sq", bufs=1) as sq_pool, \
             tc.tile_pool(name="gelu", bufs=1) as gelu_pool:
            sq = sq_pool.tile([128, KC, T], F16, tag="sq", name="sq")
            gl = gelu_pool.tile([128, FC, 512], F16, tag="gelu", name="gl")
            mus, rss = {}, {}

            def outproj_sums(hf):
                pos = [ps_op.tile([128, 512], F32, tag="po", name=f"po{m}{hf}")
                       for m in range(KC)]
                C_ORDER = [0, 1, 2, 3, 4, 5]
                for ci, c in enumerate(C_ORDER):
                    for m in range(KC):
                        nc.tensor.matmul(pos[m][:], wo[:, c, m * 128:(m + 1) * 128],
                                         attn_sb[:, c, HLF[hf]],
                                         start=(ci == 0), stop=(ci == KC - 1))
                for m in range(KC):
                    # mask is folded into the softmax reciprocal r, so the
                    # evict is one fused op: y1 = (po + bout) + x
                    nc.vector.scalar_tensor_tensor(y1[:, m, HLF[hf]], pos[m][:],
                                                   bout[:, m:m + 1],
                                                   x_big[:, m, HLF[hf]],
                                                   op0=OP.add, op1=OP.add)
                    nc.scalar.activation(sq[:, m, HLF[hf]], y1[:, m, HLF[hf]],
                                         AF.Square)
                # LN2 stats for this half
                pss = ps_s.tile([128, 512], F32, tag="sum", name=f"lns{hf}")
                for k in range(KC):
                    nc.tensor.matmul(pss[:], ones[:], y1[:, k, HLF[hf]],
                                     start=(k == 0), stop=(k == KC - 1))
                psq = ps_s.tile([128, 512], F32, tag="sum", name=f"lnq{hf}")
                for k in range(KC):
                    nc.tensor.matmul(psq[:], ones[:], sq[:, k, HLF[hf]],
                                     start=(k == 0), stop=(k == KC - 1))
                mu = stat_pool.tile([128, 512], F16, tag="mu16", name=f"mu{hf}")
                nc.vector.tensor_scalar_mul(mu[:], pss[:], 1.0 / D)
                m2 = stat_pool.tile([128, 512], F32, tag="m2", bufs=1, name=f"m2{hf}")
                nc.vector.tensor_tensor(m2[:], mu[:], mu[:], op=OP.mult)
                nc.vector.tensor_scalar_sub(m2[:], m2[:], EPS)   # var+eps below
                var = stat_pool.tile([128, 512], F32, tag="var", bufs=1, name=f"var{hf}")
                nc.vector.scalar_tensor_tensor(var[:], psq[:], 1.0 / D,
                                               m2[:], op0=OP.mult, op1=OP.subtract)
                rinv = stat_pool.tile([128, 512], F32, tag="rinv", bufs=1,
                                      name=f"rinv{hf}")
                nc.vector.reciprocal_approx_fast(out=rinv[:], in_=var[:])
                rs = stat_pool.tile([128, 512], F16, tag="rs16", name=f"rs{hf}")
                nc.scalar.activation(rs[:], rinv[:], AF.Sqrt)
                for k in range(KC):
                    nc.vector.tensor_tensor(h2[:, k, HLF[hf]], y1[:, k, HLF[hf]],
                                            mu[:], op=OP.subtract)
                    nc.vector.tensor_tensor(h2[:, k, HLF[hf]], h2[:, k, HLF[hf]],
                                            rs[:], op=OP.mult)

            outproj_sums(0)
            outproj_sums(1)
            s_op.close()
            ps_c = s_attn.enter_context(
                tc.tile_pool(name="ps_c", bufs=4, space="PSUM"))

            for hf in range(2):
                for n in range(FC):
                    pf = ps_c.tile([128, 512], F32, tag="mm", name=f"pf{n}{hf}")
                    for k in range(KC):
                        nc.tensor.matmul(pf[:], wf1[:, k, n * 128:(n + 1) * 128],
                                         h2[:, k, HLF[hf]],
                                         start=(k == 0), stop=(k == KC - 1))
                    nc.scalar.activation(gl[:, n, :], pf[:], AF.Gelu,
                                         bias=bfc1[:, n:n + 1])
                for m in range(KC):
                    py = ps_c.tile([128, 512], F32, tag="mm", name=f"py{m}{hf}")
                    for kk in range(FC):
                        nc.tensor.matmul(py[:], wf2[:, kk, m * 128:(m + 1) * 128],
                                         gl[:, kk, :],
                                         start=(kk == 0), stop=(kk == FC - 1))
                    yo = yo_pool.tile([128, 512], F16, tag="yo", name=f"yo{m}{hf}")
                    nc.vector.scalar_tensor_tensor(yo[:], py[:], bfc2[:, m:m + 1],
                                                   y1[:, m, HLF[hf]],
                                                   op0=OP.add, op1=OP.add)
                    nc.sync.dma_start(yt_d[m, :, HLF[hf]], yo[:])

        s_attn.close()

    nc.compile()
    return nc


def _get_runner():
    if "runner" in _cached:
        return _cached["runner"]
    import jax
    from jax.sharding import Mesh, PartitionSpec
    from jax.experimental.shard_map import shard_map
    import concourse.mybir as mybir
    from concourse.bass2jax import _bass_exec_p, install_neuronx_cc_hook, partition_id_tensor

    nc = _build()
    install_neuronx_cc_hook()
    partition_name = nc.partition_id_tensor.name if nc.partition_id_tensor else None
    in_names, out_names, out_avals, zero_outs = [], [], [], []
    for alloc in nc.m.functions[0].allocations:
        if not isinstance(alloc, mybir.MemoryLocationSet):
            continue
        name = alloc.memorylocations[0].name
        if alloc.kind == "ExternalInput":
            if name != partition_name:
                in_names.append(name)
        elif alloc.kind == "ExternalOutput":
            out_names.append(name)
            shape = tuple(alloc.tensor_shape)
            dtype = mybir.dt.np(alloc.dtype)
            out_avals.append(jax.core.ShapedArray(shape, dtype))
            zero_outs.append(np.zeros(shape, dtype))
    n_params = len(in_names)
    all_in_names = in_names + out_names + ([partition_name] if partition_name else [])

    def _body(*args):
        operands = list(args)
        if partition_name is not None:
            operands.append(partition_id_tensor())
        outs = _bass_exec_p.bind(
            *operands,
            out_avals=tuple(out_avals),
            in_names=tuple(all_in_names),
            out_names=tuple(out_names),
            lowering_input_output_aliases=(),
            sim_require_finite=False,
            sim_require_nnan=False,
            nc=nc,
        )
        return tuple(outs)

    donate = tuple(range(n_params, n_params + len(out_avals)))
    devices = jax.devices()[:NC]
    mesh = Mesh(np.asarray(devices), ("core",))
    in_specs = (PartitionSpec("core"),) * (n_params + len(out_avals))
    out_specs = (PartitionSpec("core"),) * len(out_names)
    sharded = jax.jit(
        shard_map(_body, mesh=mesh, in_specs=in_specs, out_specs=out_specs, check_rep=False),
        donate_argnums=donate, keep_unused=True,
    )

    runner = {
        "nc": nc, "sharded": sharded, "in_names": in_names,
        "out_names": out_names, "out_avals": out_avals, "zero_outs": zero_outs,
    }
    _cached["runner"] = runner
    return runner


def prep_inputs(x, attn_bias, node_non_padding_mask, in_w, in_b, out_w, out_b,
                ln1_g, ln1_b, fc1_w, fc1_b, fc2_w, fc2_b, ln2_g, ln2_b):
    f16, f32 = np.float16, np.float32
    x = np.asarray(x, f32)
    # LN1 on host (no affine; gamma/beta folded into qkv weights/bias)
    mu = x.mean(-1, keepdims=True)
    var = ((x - mu) ** 2).mean(-1, keepdims=True)
    h1 = (x - mu) / np.sqrt(var + EPS)
    xt = x.transpose(2, 1, 0).reshape(D, N_GRAPH * N_NODE).astype(f16)
    h1t = h1.transpose(2, 1, 0).reshape(D, N_GRAPH * N_NODE).astype(f16)
    xt_pc = [np.ascontiguousarray(xt[:, c * T:(c + 1) * T]).reshape(KC, 128, T) for c in range(NC)]
    h1t_pc = [np.ascontiguousarray(h1t[:, c * T:(c + 1) * T]).reshape(KC, 128, T) for c in range(NC)]
    # exp(bias), transposed per head-graph, key-tiled:
    # ebh[gh, p, kt*512+n] = exp(bias[gh, n, kt*128+p])
    ebt = np.exp(np.asarray(attn_bias, f32)).transpose(0, 2, 1)  # [gh, m, n]
    ebh = np.ascontiguousarray(
        ebt.reshape(N_GRAPH * H, NQT, 128, N_NODE).transpose(0, 2, 1, 3)
    ).reshape(N_GRAPH * H, 128, NQT * N_NODE).astype(f16)
    mask = np.asarray(node_non_padding_mask).astype(f16)

    in_w = np.asarray(in_w, f32)
    in_b = np.asarray(in_b, f32)
    g1 = np.asarray(ln1_g, f32)
    b1 = np.asarray(ln1_b, f32)
    wqkv = in_w * g1[None, :]                  # fold LN1 gamma
    bqkv = in_b + in_w @ b1                    # fold LN1 beta
    scale = HD ** -0.5
    wqkv = wqkv.copy(); bqkv = bqkv.copy()
    wqkv[:D] *= scale                           # fold q scaling
    bqkv[:D] *= scale
    # v bias folded into the out-projection bias: out_w @ bv + out_b
    bv = bqkv[2 * D:]
    bout_full = np.asarray(out_b, f32) + np.asarray(out_w, f32) @ bv
    fc1w = np.asarray(fc1_w, f32)
    fc1b = np.asarray(fc1_b, f32)
    g2 = np.asarray(ln2_g, f32)
    b2 = np.asarray(ln2_b, f32)
    wfc1 = fc1w * g2[None, :]                  # fold LN2 gamma
    bfc1 = fc1b + fc1w @ b2                    # fold LN2 beta

    pm = lambda a, nchunk: np.ascontiguousarray(
        a.reshape(nchunk, 128, a.shape[-1]).transpose(1, 0, 2))  # partition-major
    wqkvT = wqkv.T.astype(f16)                 # [D, 3D]
    shared = {
        "wqk": np.ascontiguousarray(wqkvT[:, :2 * D]).reshape(KC, 128, 2 * D),
        "wv": pm(np.ascontiguousarray(wqkvT[:, 2 * D:]), KC),
        "bqk": np.ascontiguousarray(bqkv[:2 * D].reshape(2 * H, HD).T),
        "wout": pm(np.asarray(out_w, f32).T.astype(f16), KC),
        "bout": np.ascontiguousarray(bout_full.reshape(KC, 128).T),
        "wfc1": pm(wfc1.T.astype(f16), KC),
        "bfc1": np.ascontiguousarray(bfc1.reshape(FC, 128).T),
        "wfc2": pm(np.asarray(fc2_w, f32).T.astype(f16), FC),
        "bfc2": np.ascontiguousarray(np.asarray(fc2_b, f32).reshape(KC, 128).T),
        "ones": np.ones((128, 128), dtype=f16),
    }
    per_core = []
    for c in range(NC):
        m = dict(shared)
        m["xt"] = np.ascontiguousarray(xt_pc[c].transpose(1, 0, 2))
        m["h1t"] = h1t_pc[c]
        m["ebias"] = ebh[G * H * c: G * H * (c + 1)]
        m["maskrow"] = np.ascontiguousarray(mask[G * c: G * (c + 1)]).reshape(1, T)
        per_core.append(m)
    return per_core


def postprocess(outs):
    yt = np.stack([np.asarray(o["yt"], np.float32).reshape(D, T) for o in outs])
    y = yt.reshape(NC, D, G, N_NODE).transpose(3, 0, 2, 1).reshape(N_NODE, N_GRAPH, D)
    return np.ascontiguousarray(y)


def run_per_core(per_core):
    r = _get_runner()
    n = NC
    concat_in = [
        np.concatenate([np.asarray(per_core[c][name]) for c in range(n)], axis=0)
        for name in r["in_names"]
    ]
    concat_zeros = [np.zeros((n * z.shape[0], *z.shape[1:]), z.dtype) for z in r["zero_outs"]]
    out_arrs = r["sharded"](*concat_in, *concat_zeros)
    return [
        {name: np.asarray(out_arrs[i]).reshape(n, *r["out_avals"][i].shape)[c]
         for i, name in enumerate(r["out_names"])}
        for c in range(n)
    ]


def kernel(**inputs):
    per_core = prep_inputs(**inputs)
    outs = run_per_core(per_core)
    return postprocess(outs)
